# revision 9
# baseline (speedup 1.0000x reference)
"""Banded causal self-attention (sparse_attention) for 8 trn2 NeuronCores.

Sharding: tensor-parallel over head groups (4 groups x 4 heads of dim 64)
x data-parallel over batch (2). Core c handles batch c//4, head group c%4.
Each core computes a partial output projection; the host sums the 4 group
partials per batch.

Layout: x is transposed on the host so every matmul on device uses natural
(pre-transposed) operands:
  qkT[512, T]   = W_qk.T @ x.T      (lhsT = W_qk natural, rhs = xT)
  v[T, 256]     = x @ W_v           (lhsT = xT natural,   rhs = W_v)
  scoresT[tk,tq]  computed as lhsT=kT_pair rhs=qpad  (K=128, zero-padded)
  yT+sums       = lhsT=[v|1|0] rhs=exp(scoresT)  (row 64 = softmax denom)
  out[T, C]     = lhsT=yTpair rhs=W_p pair rows (K=128, 2 pairs)
Softmax skips max-subtraction (scores ~ N(0,1) after 1/8 scale; exp is safe
in fp32), so the partition-dim reduction is a fused ones-column in the
att@v matmul.

HAM note: trn2's PE clock gate only un-throttles (1.2 -> 2.4 GHz) when the
MAC-activity over a ~3.4us window is high enough.  K=64 score matmuls and
M=65 att@v matmuls use half the array and left the whole attention phase
cold (measured: 58us at K=4/8).  Fix: q is stored in per-head zero-padded
[128, T] lanes so score matmuls contract K=128 (the other head's k rows
multiply exact zeros), and v is padded to 128 columns so att@v runs M=128.
Same cycle count, full MAC activity, PE stays at 2.4 GHz.

Phases A (qkT) and B (v) are fused into 4 column passes that stream the x
chunks as their DMAs land (x is split across DMA queues by partition
range; weights ride the gpsimd queue chunk-by-chunk), keeping the PE fed
during the load window.
"""

import numpy as np

B, T, C = 2, 2048, 1024
N_HEAD = 16
MEMORY = 256
D = 64           # head dim
G = 4            # head groups (tensor parallel)
HPG = 4          # heads per group
GC = HPG * D     # 256 columns per group
N_CORES = 8
TB = T // 128    # 16 row blocks
SB = T // 256    # 8 query super-blocks
D2 = 128         # padded v columns (64 v + 1 ones + 63 zero)

_PROGRAM_CACHE = {}


def _emit(tc, nc, xT, wqkv, wp, ones_in, out):
    import concourse.mybir as mybir

    f32 = mybir.dt.float32
    mmdt = mybir.dt.bfloat16

    from contextlib import ExitStack

    ctx = ExitStack()
    with ctx:
        const = ctx.enter_context(tc.tile_pool(name="const", bufs=1))
        wpool = ctx.enter_context(tc.tile_pool(name="wpool", bufs=1))
        arena = ctx.enter_context(tc.tile_pool(name="arena", bufs=9))
        qkt_pool = ctx.enter_context(tc.tile_pool(name="qkt", bufs=1))
        vplus_pool = ctx.enter_context(tc.tile_pool(name="vplus", bufs=1))
        expst_pool = ctx.enter_context(tc.tile_pool(name="expst", bufs=4))
        outsb_pool = ctx.enter_context(tc.tile_pool(name="outsb", bufs=4))
        ps1 = ctx.enter_context(tc.tile_pool(name="ps1", bufs=2, space="PSUM"))
        ps2 = ctx.enter_context(tc.tile_pool(name="ps2", bufs=3, space="PSUM"))

        # ---- constants / masks ----
        ones_sb = const.tile([128, 64], mmdt, name="ones_sb", tag="ones_sb")

        # two side-by-side lower-triangular keep masks (for the DVE mask path)
        lo2 = const.tile([128, 256], mmdt, name="lo2", tag="lo2")
        lo2_view = lo2.rearrange("p (b j) -> p b j", b=2, j=128)
        nc.vector.memset(lo2[:], 1.0)
        nc.gpsimd.affine_select(
            out=lo2_view, in_=lo2_view,
            compare_op=mybir.AluOpType.is_ge, fill=0.0,
            base=0, pattern=[[0, 2], [-1, 128]], channel_multiplier=1,
        )

        # ---- input tiles ----
        xT_sb, wqkv_sb = [], []
        for k in range(8):
            xT_sb.append(arena.tile([128, T], mmdt, name=f"xT{k}", tag="arena"))
            wqkv_sb.append(wpool.tile([128, 3 * GC], mmdt, name=f"wqkv{k}",
                                      tag=f"wqkv{k}"))
        wqk_sb = [t[:, 0:2 * GC] for t in wqkv_sb]
        wv_sb = [t[:, 2 * GC:3 * GC] for t in wqkv_sb]

        # q in zero-padded per-head lanes: qpad[pr][:, hh, :] holds head
        # (2pr+hh)'s qT rows at partition base hh*64, zeros elsewhere, so
        # score matmuls can contract K=128 (full PE rows -> HAM stays warm)
        qpad_sb = [
            qkt_pool.tile([128, 2, T], mmdt, name=f"qpad{pr}", tag=f"qpad{pr}")
            for pr in range(2)
        ]
        for pr in range(2):
            nc.vector.memset(qpad_sb[pr][0:64, 1, :], 0.0)
            nc.vector.memset(qpad_sb[pr][64:128, 0, :], 0.0)
        # k stays pair-stacked: kT[pr] rows hh*64.. hold head (2pr+hh)'s kT
        kT_sb = [
            qkt_pool.tile([128, T], mmdt, name=f"kT{pr}", tag=f"kT{pr}")
            for pr in range(2)
        ]
        # v for all 16 row blocks; per (tb, h): cols 0:64 v, col 64 ones
        # (softmax denom), cols 65:128 zero (M=128 full-array att@v)
        vp = vplus_pool.tile([128, TB, HPG, D2], mmdt, name="vplus",
                             tag="vplus")
        nc.vector.memset(vp[:, :, :, D + 1:D2], 0.0)
        nc.vector.memset(vp[:, :, :, D:D + 1], 1.0)

        # ---- input DMAs: whole chunks alternate between the two HW DGE
        # queues (one shared DGE processor serves all queues; big descriptors
        # on 2 queues empirically hit ~230GB/s, finer splits throttle) ----
        for k in range(8):
            qa = nc.sync if k % 2 == 0 else nc.scalar
            qb = nc.scalar if k % 2 == 0 else nc.sync
            qa.dma_start(xT_sb[k][:], xT[k * 128:(k + 1) * 128, :])
            qb.dma_start(wqkv_sb[k][:], wqkv[k * 128:(k + 1) * 128, :])
        wp_sb = []
        for pr in range(2):
            t = wpool.tile([128, C], mmdt, name=f"wp{pr}", tag=f"wp{pr}")
            nc.gpsimd.dma_start(t[:], wp[pr * 128:(pr + 1) * 128, :])
            wp_sb.append(t)
        nc.sync.dma_start(ones_sb[:], ones_in[:, 0:64])

        # ---- phases A+B fused: 4 column passes streaming the x chunks ----
        for t4 in range(4):
            pa = [ps2.tile([128, 1024], f32, name="psA", tag="st")
                  for _ in range(2)]
            psA = [pa[m // 2][:, (m % 2) * 512:(m % 2 + 1) * 512]
                   for m in range(4)]
            pb = [ps1.tile([128, 512], f32, name="psB", tag="ps1")
                  for _ in range(2)]
            pbx = ps2.tile([128, 1024], f32, name="psBx", tag="st")
            psB = [t[:, 0:256] for t in pb]
            psB += [pbx[:, 0:256], pbx[:, 512:768]]
            tbs = list(range(t4 * 4, t4 * 4 + 4))
            for k in range(8):
                for m in range(4):
                    nc.tensor.matmul(
                        psA[m],
                        wqk_sb[k][:, m * 128:(m + 1) * 128],
                        xT_sb[k][:, t4 * 512:(t4 + 1) * 512],
                        start=(k == 0),
                        stop=(k == 7),
                    )
                for i, tb in enumerate(tbs):
                    nc.tensor.matmul(
                        psB[i],
                        xT_sb[k][:, tb * 128:(tb + 1) * 128],
                        wv_sb[k][:],
                        start=(k == 0),
                        stop=(k == 7),
                    )
            sl = slice(t4 * 512, (t4 + 1) * 512)
            # q: split the pair psum into per-head padded lanes
            for pr in range(2):
                for hh in range(2):
                    r0 = hh * 64
                    nc.scalar.copy(
                        qpad_sb[pr][r0:r0 + 64, hh, sl],
                        psA[pr][r0:r0 + 64, :],
                    )
            for pr in range(2):
                nc.scalar.copy(kT_sb[pr][:, sl], psA[2 + pr])
            for i, tb in enumerate(tbs):
                nc.vector.tensor_copy(
                    vp[:, tb, :, 0:D],
                    psB[i].rearrange("p (h d) -> p h d", h=HPG),
                )

        # ---- phases C/D/E fused into one per-sb streaming pipeline ----
        ytn_sb = []
        for pr in range(2):
            t = arena.tile([128, T], mmdt, name=f"ytn{pr}", tag=f"ytn{pr}", bufs=1)
            ytn_sb.append(t)
        # rt: denominator bounce tile per half-T; rows (pr*2+hh)*8 .. +8
        # hold head (2pr+hh)'s denominators for that half
        rt_sb = [
            const.tile([32, 128], f32, name=f"rt{hf}", tag=f"rt{hf}")
            for hf in range(2)
        ]
        rtb_sb = [
            const.tile([32, 128], mmdt, name=f"rtb{hf}", tag=f"rtb{hf}")
            for hf in range(2)
        ]
        # pair-stacked unnormalized y (bases 0/64), denom rows separate
        ytp_sb = [
            arena.tile([128, T], f32, name=f"ytp{p}", tag=f"ytp{p}", bufs=1)
            for p in range(2)
        ]
        dn_sb = [
            [const.tile([1, T], f32, name=f"dn{p}_{hh}", tag=f"dn{p}_{hh}")
             for hh in range(2)]
            for p in range(2)
        ]
        rrp_sb = [
            const.tile([2, T], mmdt, name=f"rrp{p}", tag=f"rrp{p}")
            for p in range(2)
        ]
        # K=2 indicator: row 0 -> out partitions 0:64, row 1 -> 64:128
        ind2 = const.tile([2, 128], mmdt, name="ind2", tag="ind2")
        nc.vector.memset(ind2[:], 1.0)
        nc.gpsimd.affine_select(
            out=ind2[:], in_=ind2[:],
            compare_op=mybir.AluOpType.is_ge, fill=0.0,
            base=0, pattern=[[1, 128]], channel_multiplier=-64,
        )
        nc.gpsimd.affine_select(
            out=ind2[:], in_=ind2[:],
            compare_op=mybir.AluOpType.is_ge, fill=0.0,
            base=63, pattern=[[-1, 128]], channel_multiplier=64,
        )

        def roles_for(sb):
            roles = []
            for dtk in (-2, -1, 0, 1):
                tkb = 2 * sb + dtk
                if 0 <= tkb:
                    roles.append((tkb, "abcd"[dtk + 2]))
            return roles

        # C score layout per head, 768 cols of the expst tile:
        #   a @ [0:128)    key blk 2sb-2, queries 0:128 of sb, lower-tri keep
        #   b @ [128:384)  key blk 2sb-1, all queries; right half lower-tri
        #   c @ [384:640)  key blk 2sb,   all queries; left half upper-tri
        #   d @ [640:768)  key blk 2sb+1, queries 128:256, upper-tri keep
        # st psum packs one role GROUP for BOTH heads per tile (bank0 = h0,
        # bank1 = h1) so each exp is one strided 2-block activation call:
        #   ab tile: h0 a@0,b@128..384 | h1 a@512,b@640..896
        #   cd tile: h0 c@0..256,d@256..384 | h1 @512..896
        def emit_C(pr, sb):
            heads = (2 * pr, 2 * pr + 1)
            roles = roles_for(sb)
            n = len(roles)
            goff = {"a": (0, 128), "b": (128, 384), "c": (0, 256),
                    "d": (256, 384)}
            eoff = {"a": (0, 128), "b": (128, 384), "c": (384, 640),
                    "d": (640, 768)}
            qoff = {"a": (0, 128), "b": (0, 256), "c": (0, 256),
                    "d": (128, 256)}
            st_ab = (ps2.tile([128, 1024], f32, name="stab", tag="st")
                     if n == 4 else None)
            st_cd = ps2.tile([128, 1024], f32, name="stcd", tag="st")
            grp = {"a": st_ab, "b": st_ab, "c": st_cd, "d": st_cd}
            # K=128 zero-padded scores: lhsT = full kT pair block (the
            # other head's k rows hit qpad's zero rows -> exact 0)
            for i, (tkb, role) in enumerate(roles):
                for hh, h in enumerate(heads):
                    c0, c1 = goff[role]
                    q0, q1 = qoff[role]
                    nc.tensor.matmul(
                        grp[role][:, hh * 512 + c0:hh * 512 + c1],
                        kT_sb[pr][:, tkb * 128:(tkb + 1) * 128],
                        qpad_sb[pr][:, hh, sb * 256 + q0:sb * 256 + q1],
                        start=(i % 2 == 0),
                        stop=(i % 2 == 1 or i == n - 1),
                    )
            ep = expst_pool.tile([128, 2, 768], mmdt, name="expst",
                                 tag="expst")
            # one exp per role group covering both heads (strided 2-block)
            if n == 4:
                nc.scalar.activation(
                    ep[:, :, 0:384],
                    st_ab.rearrange("p (h j) -> p h j", h=2, j=512)[:, :, 0:384],
                    mybir.ActivationFunctionType.Exp,
                    scale=0.125,
                )
            nc.scalar.activation(
                ep[:, :, 384:768],
                st_cd.rearrange("p (h j) -> p h j", h=2, j=512)[:, :, 0:384],
                mybir.ActivationFunctionType.Exp,
                scale=0.125,
            )
            # banded mask via affine_select on GpSimd (per head, 2 blocks):
            for hh in range(2):
                if n == 4:
                    # lower-tri keep (zero where query < key) on blocks a@0
                    # and b_right@256
                    dv = ep[:, hh, 0:512].rearrange(
                        "p (x j) -> p x j", x=2, j=256)[:, :, 0:128]
                    nc.gpsimd.affine_select(
                        out=dv, in_=dv,
                        compare_op=mybir.AluOpType.is_ge, fill=0.0,
                        base=0, pattern=[[0, 2], [-1, 128]],
                        channel_multiplier=1,
                    )
                # upper-tri keep (zero where key more than MEMORY behind) on
                # blocks c_left@384 and d@640
                uv = ep[:, hh, 256:768].rearrange(
                    "p (x j) -> p x j", x=2, j=256)[:, :, 128:256]
                nc.gpsimd.affine_select(
                    out=uv, in_=uv,
                    compare_op=mybir.AluOpType.is_ge, fill=0.0,
                    base=0, pattern=[[0, 2], [1, 128]],
                    channel_multiplier=-1,
                )
            yts_pair = [
                ps1.tile([128, 256], f32, name="yts", tag="ps1")
                for _ in heads
            ]
            order = [r for r in roles if r[1] in "bc"] + [
                r for r in roles if r[1] in "ad"
            ]
            # M=128 padded att@v (zero v columns produce zero rows)
            for j, (tkb, role) in enumerate(order):
                for hh, h in enumerate(heads):
                    c0, c1 = eoff[role]
                    q0, q1 = qoff[role]
                    nc.tensor.matmul(
                        yts_pair[hh][:, q0:q1],
                        vp[:, tkb, h, :],
                        ep[:, hh, c0:c1],
                        start=(j == 0),
                        stop=(j == n - 1),
                    )
            for hh, h in enumerate(heads):
                yts = yts_pair[hh]
                r0 = (h % 2) * 64
                nc.vector.tensor_copy(
                    ytp_sb[pr][r0:r0 + 64, sb * 256:(sb + 1) * 256],
                    yts[0:64, :],
                )
                nc.scalar.copy(
                    dn_sb[pr][h % 2][:, sb * 256:(sb + 1) * 256],
                    yts[64:65, :],
                )

        def emit_D_recip(hf):
            # reciprocal on [1, T] is ~us on one DVE lane; bounce the half-T
            # denom rows through a [32, 128] tile with tiny SBUF->SBUF DMAs
            cs = slice(hf * 1024, (hf + 1) * 1024)
            rt = rt_sb[hf]
            for pr in range(2):
                for hh in range(2):
                    r0 = (pr * 2 + hh) * 8
                    nc.sync.dma_start(rt[r0:r0 + 8, :], dn_sb[pr][hh][:, cs])
            with nc.allow_low_precision(reason="softmax denom reciprocal"):
                nc.vector.reciprocal(rtb_sb[hf][0:32, :], rt[0:32, :])
            # [16,128] rows map linearly onto [2, 1024]: one DMA per pair
            for pr in range(2):
                nc.sync.dma_start(
                    rrp_sb[pr][0:2, cs], rtb_sb[hf][pr * 16:(pr + 1) * 16, :]
                )

        def emit_D_norm(pr, t4):
            sl = slice(t4 * 512, (t4 + 1) * 512)
            # pair-stacked broadcast: one K=2 matmul + one [128,512]
            # DVE multiply normalize both heads of the pair
            bc = ps1.tile([128, 512], f32, name="bc", tag="ps1")
            nc.tensor.matmul(
                bc[:], ind2[:, :], rrp_sb[pr][:, sl],
                start=True, stop=True,
            )
            nc.vector.tensor_mul(
                ytn_sb[pr][:, sl], ytp_sb[pr][:, sl], bc[:],
            )

        # ---- phase E: partial projection out = y_g @ W_p[g] (K=128 pairs) --
        store_q = [nc.sync, nc.scalar]

        def emit_E(tbs):
            for tb in tbs:
                ob = outsb_pool.tile([128, 1024], mmdt, name="outsb",
                                     tag="outsb")
                for nh in range(2):
                    ps = ps1.tile([128, 512], f32, name="psE", tag="ps1")
                    for pr in range(2):
                        nc.tensor.matmul(
                            ps[:],
                            ytn_sb[pr][:, tb * 128:(tb + 1) * 128],
                            wp_sb[pr][:, nh * 512:(nh + 1) * 512],
                            start=(pr == 0),
                            stop=(pr == 1),
                        )
                    if (tb + nh) % 2 == 0:
                        nc.scalar.copy(ob[:, nh * 512:(nh + 1) * 512], ps[:])
                    else:
                        nc.vector.tensor_copy(
                            ob[:, nh * 512:(nh + 1) * 512], ps[:])
                # one [128, 1024] store: 2KB rows, efficient descriptors
                qo = store_q[tb % 2]
                qo.dma_start(out[tb * 128:(tb + 1) * 128, :], ob[:])

        # per half-T: attention for all 4 sbs of both pairs, then recip,
        # then normalize + project + store those 8 row blocks while the
        # next half's attention streams
        for hf in range(2):
            for sb in range(hf * 4, hf * 4 + 4):
                emit_C(0, sb)
                emit_C(1, sb)
            emit_D_recip(hf)
            for t4 in (hf * 2, hf * 2 + 1):
                emit_D_norm(0, t4)
                emit_D_norm(1, t4)
                emit_E(range(t4 * 4, t4 * 4 + 4))


def build_program():
    key = "v17"
    if key in _PROGRAM_CACHE:
        return _PROGRAM_CACHE[key]
    import concourse.bacc as bacc
    import concourse.mybir as mybir
    import concourse.tile as tile

    mmdt = mybir.dt.bfloat16
    nc = bacc.Bacc("TRN2", target_bir_lowering=False, debug=False, num_devices=N_CORES)
    xT = nc.dram_tensor("xT", [C, T], mmdt, kind="ExternalInput").ap()
    wqkv = nc.dram_tensor("wqkv", [C, 3 * GC], mmdt, kind="ExternalInput").ap()
    wp = nc.dram_tensor("wp", [GC, C], mmdt, kind="ExternalInput").ap()
    ones_in = nc.dram_tensor("ones_in", [128, 64 + HPG], mmdt,
                             kind="ExternalInput").ap()
    out = nc.dram_tensor("out", [T, C], mmdt, kind="ExternalOutput").ap()
    with tile.TileContext(nc) as tc:
        _emit(tc, nc, xT, wqkv, wp, ones_in, out)
    nc.compile()
    _PROGRAM_CACHE[key] = nc
    return nc


def make_in_maps(x, W_attn, W_proj):
    import ml_dtypes

    x = np.asarray(x, dtype=np.float32)
    W_attn = np.asarray(W_attn, dtype=np.float32)
    W_proj = np.asarray(W_proj, dtype=np.float32)
    cast = lambda a: np.ascontiguousarray(a, dtype=ml_dtypes.bfloat16)
    xTs = [cast(x[b].T) for b in range(B)]
    in_maps = []
    for c in range(N_CORES):
        b, g = divmod(c, G)
        q_cols = W_attn[:, g * GC:(g + 1) * GC]
        k_cols = W_attn[:, C + g * GC:C + (g + 1) * GC]
        v_cols = W_attn[:, 2 * C + g * GC:2 * C + (g + 1) * GC]
        in_maps.append({
            "xT": xTs[b],
            "wqkv": cast(np.concatenate([q_cols, k_cols, v_cols], axis=1)),
            "wp": cast(W_proj[g * GC:(g + 1) * GC, :]),
            "ones_in": cast(np.ones((128, 64 + HPG), dtype=np.float32)),
        })
    return in_maps


def gather(results):
    out = np.zeros((B, T, C), dtype=np.float32)
    for c, res in enumerate(results):
        b = c // G
        out[b] += np.asarray(res["out"], dtype=np.float32)
    return out


def kernel(x, W_attn, W_proj, dtype="bf16", trace=False):
    from concourse import bass_utils

    nc = build_program()
    in_maps = make_in_maps(x, W_attn, W_proj)
    r = bass_utils.run_bass_kernel_spmd(
        nc, in_maps, core_ids=list(range(N_CORES)), trace=trace
    )
    out = gather(r.results)
    if trace:
        kernel.last_results = r
    return out


# revision 18
# speedup vs baseline: 1.0210x; 1.0210x over previous
"""Banded causal self-attention (sparse_attention) for 8 trn2 NeuronCores.

Sharding: tensor-parallel over head groups (4 groups x 4 heads of dim 64)
x data-parallel over batch (2). Core c handles batch c//4, head group c%4.
Each core computes a partial output projection; the host sums the 4 group
partials per batch.

Layout: x is transposed on the host so every matmul on device uses natural
(pre-transposed) operands:
  qkT[512, T]   = W_qk.T @ x.T      (lhsT = W_qk natural, rhs = xT)
  v[T, 256]     = x @ W_v           (lhsT = xT natural,   rhs = W_v)
  scoresT[tk,tq]  computed as lhsT=kT_pair rhs=qpad  (K=128, zero-padded)
  yT+sums       = lhsT=[v|1|0] rhs=exp(scoresT)  (row 64 = softmax denom)
  out[T, C]     = lhsT=yTpair rhs=W_p pair rows (K=128, 2 pairs)
Softmax skips max-subtraction (scores ~ N(0,1) after 1/8 scale; exp is safe
in fp32), so the partition-dim reduction is a fused ones-column in the
att@v matmul.

HAM note: trn2's PE clock gate only un-throttles (1.2 -> 2.4 GHz) when the
MAC-activity over a ~3.4us window is high enough.  K=64 score matmuls and
M=65 att@v matmuls use half the array and left the whole attention phase
cold (measured: 58us at K=4/8).  Fix: q is stored in per-head zero-padded
[128, T] lanes so score matmuls contract K=128 (the other head's k rows
multiply exact zeros), and v is padded to 128 columns so att@v runs M=128.
Same cycle count, full MAC activity, PE stays at 2.4 GHz.

Phases A (qkT) and B (v) are fused into 4 column passes that stream the x
chunks as their DMAs land (x is split across DMA queues by partition
range; weights ride the gpsimd queue chunk-by-chunk), keeping the PE fed
during the load window.
"""

import numpy as np

B, T, C = 2, 2048, 1024
N_HEAD = 16
MEMORY = 256
D = 64           # head dim
G = 4            # head groups (tensor parallel)
HPG = 4          # heads per group
GC = HPG * D     # 256 columns per group
N_CORES = 8
TB = T // 128    # 16 row blocks
SB = T // 256    # 8 query super-blocks
D2 = 128         # padded v columns (64 v + 1 ones + 63 zero)

_PROGRAM_CACHE = {}


def _emit(tc, nc, xT, wqkv, wp, ones_in, out):
    import concourse.mybir as mybir

    f32 = mybir.dt.float32
    mmdt = mybir.dt.bfloat16

    from contextlib import ExitStack

    ctx = ExitStack()
    with ctx:
        const = ctx.enter_context(tc.tile_pool(name="const", bufs=1))
        wpool = ctx.enter_context(tc.tile_pool(name="wpool", bufs=1))
        arena = ctx.enter_context(tc.tile_pool(name="arena", bufs=9))
        qkt_pool = ctx.enter_context(tc.tile_pool(name="qkt", bufs=1))
        vplus_pool = ctx.enter_context(tc.tile_pool(name="vplus", bufs=1))
        expst_pool = ctx.enter_context(tc.tile_pool(name="expst", bufs=4))
        outsb_pool = ctx.enter_context(tc.tile_pool(name="outsb", bufs=4))
        ps1 = ctx.enter_context(tc.tile_pool(name="ps1", bufs=2, space="PSUM"))
        ps2 = ctx.enter_context(tc.tile_pool(name="ps2", bufs=3, space="PSUM"))

        # ---- constants / masks ----
        # two side-by-side triangular keep masks (multiplied in after exp):
        # lo2: keep p >= j (memory-window edge), up2: keep j >= p (causal)
        lo2 = const.tile([128, 256], mmdt, name="lo2", tag="lo2")
        lo2_view = lo2.rearrange("p (b j) -> p b j", b=2, j=128)
        nc.vector.memset(lo2[:], 1.0)
        nc.gpsimd.affine_select(
            out=lo2_view, in_=lo2_view,
            compare_op=mybir.AluOpType.is_ge, fill=0.0,
            base=0, pattern=[[0, 2], [-1, 128]], channel_multiplier=1,
        )
        up2 = const.tile([128, 256], mmdt, name="up2", tag="up2")
        up2_view = up2.rearrange("p (b j) -> p b j", b=2, j=128)
        nc.vector.memset(up2[:], 1.0)
        nc.gpsimd.affine_select(
            out=up2_view, in_=up2_view,
            compare_op=mybir.AluOpType.is_ge, fill=0.0,
            base=0, pattern=[[0, 2], [1, 128]], channel_multiplier=-1,
        )

        # ---- input tiles ----
        xT_sb, wqkv_sb = [], []
        for k in range(8):
            xT_sb.append(arena.tile([128, T], mmdt, name=f"xT{k}", tag="arena"))
            wqkv_sb.append(wpool.tile([128, 3 * GC], mmdt, name=f"wqkv{k}",
                                      tag=f"wqkv{k}"))
        wqk_sb = [t[:, 0:2 * GC] for t in wqkv_sb]
        wv_sb = [t[:, 2 * GC:3 * GC] for t in wqkv_sb]

        # q in zero-padded per-head lanes: qpad[pr][:, hh, :] holds head
        # (2pr+hh)'s qT rows at partition base hh*64, zeros elsewhere, so
        # score matmuls can contract K=128 (full PE rows -> HAM stays warm)
        qpad_sb = [
            qkt_pool.tile([128, 2, T], mmdt, name=f"qpad{pr}", tag=f"qpad{pr}")
            for pr in range(2)
        ]
        for pr in range(2):
            nc.vector.memset(qpad_sb[pr][0:64, 1, :], 0.0)
            nc.vector.memset(qpad_sb[pr][64:128, 0, :], 0.0)
        # k stays pair-stacked: kT[pr] rows hh*64.. hold head (2pr+hh)'s kT
        kT_sb = [
            qkt_pool.tile([128, T], mmdt, name=f"kT{pr}", tag=f"kT{pr}")
            for pr in range(2)
        ]
        # v for all 16 row blocks; per (tb, h): cols 0:64 v, col 64 ones
        # (softmax denom), cols 65:128 zero (M=128 full-array att@v)
        vp = vplus_pool.tile([128, TB, HPG, D2], mmdt, name="vplus",
                             tag="vplus")
        nc.vector.memset(vp[:, :, :, D + 1:D2], 0.0)
        nc.vector.memset(vp[:, :, :, D:D + 1], 1.0)

        # ---- input DMAs: whole chunks alternate between the two HW DGE
        # queues (one shared DGE processor serves all queues; big descriptors
        # on 2 queues empirically hit ~230GB/s, finer splits throttle) ----
        for k in range(8):
            qa = nc.sync if k % 2 == 0 else nc.scalar
            qb = nc.scalar if k % 2 == 0 else nc.sync
            qa.dma_start(xT_sb[k][:], xT[k * 128:(k + 1) * 128, :])
            qb.dma_start(wqkv_sb[k][:], wqkv[k * 128:(k + 1) * 128, :])
        wp_sb = []
        for pr in range(2):
            t = wpool.tile([128, C], mmdt, name=f"wp{pr}", tag=f"wp{pr}")
            nc.gpsimd.dma_start(t[:], wp[pr * 128:(pr + 1) * 128, :])
            wp_sb.append(t)

        # ---- phases A+B fused: 4 column passes streaming the x chunks ----
        for t4 in range(4):
            pa = [ps2.tile([128, 1024], f32, name="psA", tag="st")
                  for _ in range(2)]
            psA = [pa[m // 2][:, (m % 2) * 512:(m % 2 + 1) * 512]
                   for m in range(4)]
            pb = [ps1.tile([128, 512], f32, name="psB", tag="ps1")
                  for _ in range(2)]
            pbx = ps2.tile([128, 1024], f32, name="psBx", tag="st")
            psB = [t[:, 0:256] for t in pb]
            psB += [pbx[:, 0:256], pbx[:, 512:768]]
            tbs = list(range(t4 * 4, t4 * 4 + 4))
            for k in range(8):
                for m in range(4):
                    nc.tensor.matmul(
                        psA[m],
                        wqk_sb[k][:, m * 128:(m + 1) * 128],
                        xT_sb[k][:, t4 * 512:(t4 + 1) * 512],
                        start=(k == 0),
                        stop=(k == 7),
                    )
                for i, tb in enumerate(tbs):
                    nc.tensor.matmul(
                        psB[i],
                        xT_sb[k][:, tb * 128:(tb + 1) * 128],
                        wv_sb[k][:],
                        start=(k == 0),
                        stop=(k == 7),
                    )
            sl = slice(t4 * 512, (t4 + 1) * 512)
            # q: split the pair psum into per-head padded lanes
            for pr in range(2):
                for hh in range(2):
                    r0 = hh * 64
                    nc.scalar.copy(
                        qpad_sb[pr][r0:r0 + 64, hh, sl],
                        psA[pr][r0:r0 + 64, :],
                    )
            for pr in range(2):
                nc.any.tensor_copy(kT_sb[pr][:, sl], psA[2 + pr])
            for i, tb in enumerate(tbs):
                nc.any.tensor_copy(
                    vp[:, tb, :, 0:D],
                    psB[i].rearrange("p (h d) -> p h d", h=HPG),
                )

        # ---- phases C/D/E fused into one per-sb streaming pipeline ----
        ytn_sb = []
        for pr in range(2):
            t = arena.tile([128, T], mmdt, name=f"ytn{pr}", tag=f"ytn{pr}", bufs=1)
            ytn_sb.append(t)
        # rt: denominator bounce tiles (ping-pong per t4); rows
        # (pr*2+hh)*4 .. +4 hold head (2pr+hh)'s denominators for that t4
        rt_sb = [
            const.tile([16, 128], f32, name=f"rt{i}", tag=f"rt{i}")
            for i in range(2)
        ]
        rtb_sb = [
            const.tile([16, 128], mmdt, name=f"rtb{i}", tag=f"rtb{i}")
            for i in range(2)
        ]
        # per-head unnormalized y.T; row 64 = softmax denominators
        yt_sb = [
            arena.tile([65, T], f32, name=f"yt{h}", tag=f"yt{h}", bufs=1)
            for h in range(HPG)
        ]
        rrp_sb = [
            const.tile([2, T], mmdt, name=f"rrp{p}", tag=f"rrp{p}")
            for p in range(2)
        ]
        # K=2 indicator: row 0 -> out partitions 0:64, row 1 -> 64:128
        ind2 = const.tile([2, 128], mmdt, name="ind2", tag="ind2")
        nc.vector.memset(ind2[:], 1.0)
        nc.gpsimd.affine_select(
            out=ind2[:], in_=ind2[:],
            compare_op=mybir.AluOpType.is_ge, fill=0.0,
            base=0, pattern=[[1, 128]], channel_multiplier=-64,
        )
        nc.gpsimd.affine_select(
            out=ind2[:], in_=ind2[:],
            compare_op=mybir.AluOpType.is_ge, fill=0.0,
            base=63, pattern=[[-1, 128]], channel_multiplier=64,
        )

        def roles_for(sb):
            roles = []
            for dtk in (-2, -1, 0, 1):
                tkb = 2 * sb + dtk
                if 0 <= tkb:
                    roles.append((tkb, "abcd"[dtk + 2]))
            return roles

        # C score layout per head, 768 cols of the expst tile:
        #   a @ [0:128)    key blk 2sb-2, queries 0:128 of sb, lower-tri keep
        #   b @ [128:384)  key blk 2sb-1, all queries; right half lower-tri
        #   c @ [384:640)  key blk 2sb,   all queries; left half upper-tri
        #   d @ [640:768)  key blk 2sb+1, queries 128:256, upper-tri keep
        # st psum packs one role GROUP for BOTH heads per tile (bank0 = h0,
        # bank1 = h1) so each exp is one strided 2-block activation call:
        #   ab tile: h0 a@0,b@128..384 | h1 a@512,b@640..896
        #   cd tile: h0 c@0..256,d@256..384 | h1 @512..896
        def emit_C(pr, sb):
            heads = (2 * pr, 2 * pr + 1)
            roles = roles_for(sb)
            n = len(roles)
            goff = {"a": (0, 128), "b": (128, 384), "c": (0, 256),
                    "d": (256, 384)}
            eoff = {"a": (0, 128), "b": (128, 384), "c": (384, 640),
                    "d": (640, 768)}
            qoff = {"a": (0, 128), "b": (0, 256), "c": (0, 256),
                    "d": (128, 256)}
            st_ab = (ps2.tile([128, 1024], f32, name="stab", tag="st")
                     if n == 4 else None)
            st_cd = ps2.tile([128, 1024], f32, name="stcd", tag="st")
            grp = {"a": st_ab, "b": st_ab, "c": st_cd, "d": st_cd}
            # K=128 zero-padded scores: lhsT = full kT pair block (the
            # other head's k rows hit qpad's zero rows -> exact 0)
            for i, (tkb, role) in enumerate(roles):
                for hh, h in enumerate(heads):
                    c0, c1 = goff[role]
                    q0, q1 = qoff[role]
                    nc.tensor.matmul(
                        grp[role][:, hh * 512 + c0:hh * 512 + c1],
                        kT_sb[pr][:, tkb * 128:(tkb + 1) * 128],
                        qpad_sb[pr][:, hh, sb * 256 + q0:sb * 256 + q1],
                        start=(i % 2 == 0),
                        stop=(i % 2 == 1 or i == n - 1),
                    )
            ep = expst_pool.tile([128, 2, 768], mmdt, name="expst",
                                 tag="expst")
            # one exp per role group covering both heads (strided 2-block)
            if n == 4:
                nc.scalar.activation(
                    ep[:, :, 0:384],
                    st_ab.rearrange("p (h j) -> p h j", h=2, j=512)[:, :, 0:384],
                    mybir.ActivationFunctionType.Exp,
                    scale=0.125,
                )
            nc.scalar.activation(
                ep[:, :, 384:768],
                st_cd.rearrange("p (h j) -> p h j", h=2, j=512)[:, :, 0:384],
                mybir.ActivationFunctionType.Exp,
                scale=0.125,
            )
            # banded mask: multiply const triangle tiles in after exp
            # (scheduler-balanced across DVE/GpSimd)
            for hh in range(2):
                if n == 4:
                    # keep p >= j (memory-window edge) on blocks a@0 and
                    # b_right@256
                    dv = ep[:, hh, 0:512].rearrange(
                        "p (x j) -> p x j", x=2, j=256)[:, :, 0:128]
                    nc.any.tensor_mul(dv, dv, lo2_view)
                # keep j >= p (causal) on blocks c_left@384 and d@640
                uv = ep[:, hh, 256:768].rearrange(
                    "p (x j) -> p x j", x=2, j=256)[:, :, 128:256]
                nc.any.tensor_mul(uv, uv, up2_view)
            yts_pair = [
                ps1.tile([128, 256], f32, name="yts", tag="ps1")
                for _ in heads
            ]
            order = [r for r in roles if r[1] in "bc"] + [
                r for r in roles if r[1] in "ad"
            ]
            # M=128 padded att@v (zero v columns produce zero rows)
            for j, (tkb, role) in enumerate(order):
                for hh, h in enumerate(heads):
                    c0, c1 = eoff[role]
                    q0, q1 = qoff[role]
                    nc.tensor.matmul(
                        yts_pair[hh][:, q0:q1],
                        vp[:, tkb, h, :],
                        ep[:, hh, c0:c1],
                        start=(j == 0),
                        stop=(j == n - 1),
                    )
            for hh, h in enumerate(heads):
                nc.any.tensor_copy(
                    yt_sb[h][:, sb * 256:(sb + 1) * 256],
                    yts_pair[hh][0:65, :],
                )

        def emit_D_recip(t4):
            # reciprocal on [1, T] is ~us on one DVE lane; bounce the t4's
            # denom rows through a [16, 128] tile with tiny SBUF->SBUF DMAs
            cs = slice(t4 * 512, (t4 + 1) * 512)
            rt = rt_sb[t4 % 2]
            rtb = rtb_sb[t4 % 2]
            for h in range(HPG):
                r0 = h * 4
                nc.sync.dma_start(rt[r0:r0 + 4, :], yt_sb[h][64:65, cs])
            with nc.allow_low_precision(reason="softmax denom reciprocal"):
                nc.vector.reciprocal(rtb[0:16, :], rt[0:16, :])
            # [8,128] rows map linearly onto [2, 512]: one DMA per pair
            for pr in range(2):
                nc.sync.dma_start(
                    rrp_sb[pr][0:2, cs], rtb[pr * 8:(pr + 1) * 8, :]
                )

        def emit_D_norm(pr, t4):
            sl = slice(t4 * 512, (t4 + 1) * 512)
            # pair-stacked broadcast: one K=2 matmul broadcasts both heads'
            # reciprocal rows across partitions, then per-head multiplies
            bc = ps1.tile([128, 512], f32, name="bc", tag="ps1")
            nc.tensor.matmul(
                bc[:], ind2[:, :], rrp_sb[pr][:, sl],
                start=True, stop=True,
            )
            for hh in range(2):
                r0 = hh * 64
                nc.any.tensor_mul(
                    ytn_sb[pr][r0:r0 + 64, sl],
                    yt_sb[2 * pr + hh][0:64, sl],
                    bc[r0:r0 + 64, :],
                )

        # ---- phase E: partial projection out = y_g @ W_p[g] (K=128 pairs) --
        store_q = [nc.sync, nc.scalar]

        def emit_E(tbs):
            for tb in tbs:
                ob = outsb_pool.tile([128, 1024], mmdt, name="outsb",
                                     tag="outsb")
                for nh in range(2):
                    ps = ps1.tile([128, 512], f32, name="psE", tag="ps1")
                    for pr in range(2):
                        nc.tensor.matmul(
                            ps[:],
                            ytn_sb[pr][:, tb * 128:(tb + 1) * 128],
                            wp_sb[pr][:, nh * 512:(nh + 1) * 512],
                            start=(pr == 0),
                            stop=(pr == 1),
                        )
                    if (tb + nh) % 2 == 0:
                        nc.scalar.copy(ob[:, nh * 512:(nh + 1) * 512], ps[:])
                    else:
                        nc.any.tensor_copy(
                            ob[:, nh * 512:(nh + 1) * 512], ps[:])
                # one [128, 1024] store: 2KB rows, efficient descriptors
                qo = store_q[tb % 2]
                qo.dma_start(out[tb * 128:(tb + 1) * 128, :], ob[:])

        # per quarter-T: attention for both pairs of 2 sbs, then recip,
        # then normalize + project + store those 4 row blocks while the
        # next quarter's attention streams
        for t4 in range(4):
            for sb in (t4 * 2, t4 * 2 + 1):
                emit_C(0, sb)
                emit_C(1, sb)
            emit_D_recip(t4)
            emit_D_norm(0, t4)
            emit_D_norm(1, t4)
            emit_E(range(t4 * 4, t4 * 4 + 4))


def build_program():
    key = "v18"
    if key in _PROGRAM_CACHE:
        return _PROGRAM_CACHE[key]
    import concourse.bacc as bacc
    import concourse.mybir as mybir
    import concourse.tile as tile

    mmdt = mybir.dt.bfloat16
    nc = bacc.Bacc("TRN2", target_bir_lowering=False, debug=False, num_devices=N_CORES)
    xT = nc.dram_tensor("xT", [C, T], mmdt, kind="ExternalInput").ap()
    wqkv = nc.dram_tensor("wqkv", [C, 3 * GC], mmdt, kind="ExternalInput").ap()
    wp = nc.dram_tensor("wp", [GC, C], mmdt, kind="ExternalInput").ap()
    ones_in = nc.dram_tensor("ones_in", [128, 64 + HPG], mmdt,
                             kind="ExternalInput").ap()
    out = nc.dram_tensor("out", [T, C], mmdt, kind="ExternalOutput").ap()
    with tile.TileContext(nc) as tc:
        _emit(tc, nc, xT, wqkv, wp, ones_in, out)
    nc.compile()
    _PROGRAM_CACHE[key] = nc
    return nc


def make_in_maps(x, W_attn, W_proj):
    import ml_dtypes

    x = np.asarray(x, dtype=np.float32)
    W_attn = np.asarray(W_attn, dtype=np.float32)
    W_proj = np.asarray(W_proj, dtype=np.float32)
    cast = lambda a: np.ascontiguousarray(a, dtype=ml_dtypes.bfloat16)
    xTs = [cast(x[b].T) for b in range(B)]
    in_maps = []
    for c in range(N_CORES):
        b, g = divmod(c, G)
        q_cols = W_attn[:, g * GC:(g + 1) * GC]
        k_cols = W_attn[:, C + g * GC:C + (g + 1) * GC]
        v_cols = W_attn[:, 2 * C + g * GC:2 * C + (g + 1) * GC]
        in_maps.append({
            "xT": xTs[b],
            "wqkv": cast(np.concatenate([q_cols, k_cols, v_cols], axis=1)),
            "wp": cast(W_proj[g * GC:(g + 1) * GC, :]),
            "ones_in": cast(np.ones((128, 64 + HPG), dtype=np.float32)),
        })
    return in_maps


def gather(results):
    out = np.zeros((B, T, C), dtype=np.float32)
    for c, res in enumerate(results):
        b = c // G
        out[b] += np.asarray(res["out"], dtype=np.float32)
    return out


def kernel(x, W_attn, W_proj, dtype="bf16", trace=False):
    from concourse import bass_utils

    nc = build_program()
    in_maps = make_in_maps(x, W_attn, W_proj)
    r = bass_utils.run_bass_kernel_spmd(
        nc, in_maps, core_ids=list(range(N_CORES)), trace=trace
    )
    out = gather(r.results)
    if trace:
        kernel.last_results = r
    return out


# revision 34
# speedup vs baseline: 1.0513x; 1.0297x over previous
"""Banded causal self-attention (sparse_attention) for 8 trn2 NeuronCores.

Sharding: tensor-parallel over head groups (4 groups x 4 heads of dim 64)
x data-parallel over batch (2). Core c handles batch c//4, head group c%4.
Each core computes a partial output projection; the host sums the 4 group
partials per batch.

Layout: x is transposed on the host so every matmul on device uses natural
(pre-transposed) operands:
  qkT[512, T]   = W_qk.T @ x.T      (lhsT = W_qk natural, rhs = xT)
  v[T, 256]     = x @ W_v           (lhsT = xT natural,   rhs = W_v)
  scoresT[tk,tq]  computed as lhsT=kT_pair rhs=qpad  (K=128, zero-padded)
  yT+sums       = lhsT=[v|1|0] rhs=exp(scoresT)  (row 64 = softmax denom)
  out[T, C]     = lhsT=yTpair rhs=W_p pair rows (K=128, 2 pairs)
Softmax skips max-subtraction (scores ~ N(0,1) after 1/8 scale; exp is safe
in fp32), so the partition-dim reduction is a fused ones-column in the
att@v matmul.

HAM note: trn2's PE clock gate only un-throttles (1.2 -> 2.4 GHz) when the
MAC-activity over a ~3.4us window is high enough.  K=64 score matmuls and
M=65 att@v matmuls use half the array and left the whole attention phase
cold (measured: 58us at K=4/8).  Fix: q is stored in per-head zero-padded
[128, T] lanes so score matmuls contract K=128 (the other head's k rows
multiply exact zeros), and v is padded to 128 columns so att@v runs M=128.
Same cycle count, full MAC activity, PE stays at 2.4 GHz.

Phases A (qkT) and B (v) are fused into 4 column passes that stream the x
chunks as their DMAs land (x is split across DMA queues by partition
range; weights ride the gpsimd queue chunk-by-chunk), keeping the PE fed
during the load window.
"""

import numpy as np

B, T, C = 2, 2048, 1024
N_HEAD = 16
MEMORY = 256
D = 64           # head dim
G = 4            # head groups (tensor parallel)
HPG = 4          # heads per group
GC = HPG * D     # 256 columns per group
N_CORES = 8
TB = T // 128    # 16 row blocks
SB = T // 256    # 8 query super-blocks
D2 = 128         # padded v columns (64 v + 1 ones + 63 zero)

_PROGRAM_CACHE = {}


def _emit(tc, nc, xT, wqkv, wp, ones_in, out):
    import concourse.mybir as mybir

    f32 = mybir.dt.float32
    mmdt = mybir.dt.bfloat16

    from contextlib import ExitStack

    ctx = ExitStack()
    with ctx:
        const = ctx.enter_context(tc.tile_pool(name="const", bufs=1))
        wpool = ctx.enter_context(tc.tile_pool(name="wpool", bufs=1))
        arena = ctx.enter_context(tc.tile_pool(name="arena", bufs=9))
        qkt_pool = ctx.enter_context(tc.tile_pool(name="qkt", bufs=1))
        vplus_pool = ctx.enter_context(tc.tile_pool(name="vplus", bufs=1))
        expst_pool = ctx.enter_context(tc.tile_pool(name="expst", bufs=4))
        outsb_pool = ctx.enter_context(tc.tile_pool(name="outsb", bufs=4))
        # PSUM (8 banks x 2KB): ps2 = 2x[128,1024] (psA pair / ab+cd score
        # tiles), ps1 = 2x[128,512] (B-chains 0,1 / av yts pair), psd =
        # 2x[128,512] (B-chains 2,3 / bc broadcast + projection psE).
        # yts and bc/psE live in different pools so a reciprocal-latency
        # stall on bc can't head-of-line-block the next quarter's av.
        ps1 = ctx.enter_context(tc.tile_pool(name="ps1", bufs=2, space="PSUM"))
        ps2 = ctx.enter_context(tc.tile_pool(name="ps2", bufs=2, space="PSUM"))
        psd = ctx.enter_context(tc.tile_pool(name="psd", bufs=2, space="PSUM"))

        # ---- constants / masks ----
        # two side-by-side triangular keep masks (multiplied in after exp):
        # lo2: keep p >= j (memory-window edge), up2: keep j >= p (causal)
        lo2 = const.tile([128, 256], mmdt, name="lo2", tag="lo2")
        lo2_view = lo2.rearrange("p (b j) -> p b j", b=2, j=128)
        nc.vector.memset(lo2[:], 1.0)
        nc.gpsimd.affine_select(
            out=lo2_view, in_=lo2_view,
            compare_op=mybir.AluOpType.is_ge, fill=0.0,
            base=0, pattern=[[0, 2], [-1, 128]], channel_multiplier=1,
        )
        up2 = const.tile([128, 256], mmdt, name="up2", tag="up2")
        up2_view = up2.rearrange("p (b j) -> p b j", b=2, j=128)
        nc.vector.memset(up2[:], 1.0)
        nc.gpsimd.affine_select(
            out=up2_view, in_=up2_view,
            compare_op=mybir.AluOpType.is_ge, fill=0.0,
            base=0, pattern=[[0, 2], [1, 128]], channel_multiplier=-1,
        )

        # ---- input tiles ----
        xT_sb, wqkv_sb = [], []
        for k in range(8):
            xT_sb.append(arena.tile([128, T], mmdt, name=f"xT{k}", tag="arena"))
            wqkv_sb.append(wpool.tile([128, 3 * GC], mmdt, name=f"wqkv{k}",
                                      tag=f"wqkv{k}"))
        wqk_sb = [t[:, 0:2 * GC] for t in wqkv_sb]
        wv_sb = [t[:, 2 * GC:3 * GC] for t in wqkv_sb]

        # q in zero-padded per-head lanes: qpad[pr][:, hh, :] holds head
        # (2pr+hh)'s qT rows at partition base hh*64, zeros elsewhere, so
        # score matmuls can contract K=128 (full PE rows -> HAM stays warm)
        qpad_sb = [
            qkt_pool.tile([128, 2, T], mmdt, name=f"qpad{pr}", tag=f"qpad{pr}")
            for pr in range(2)
        ]
        for pr in range(2):
            nc.vector.memset(qpad_sb[pr][0:64, 1, :], 0.0)
            nc.vector.memset(qpad_sb[pr][64:128, 0, :], 0.0)
        # k stays pair-stacked: kT[pr] rows hh*64.. hold head (2pr+hh)'s kT
        kT_sb = [
            qkt_pool.tile([128, T], mmdt, name=f"kT{pr}", tag=f"kT{pr}")
            for pr in range(2)
        ]
        # v for all 16 row blocks; per (tb, h): cols 0:64 v, col 64 ones
        # (softmax denom), cols 65:128 zero (M=128 full-array att@v)
        vp = vplus_pool.tile([128, TB, HPG, D2], mmdt, name="vplus",
                             tag="vplus")
        # whole-tile memset (contiguous, fast), then the ones column
        nc.vector.memset(vp[:], 0.0)
        nc.vector.memset(vp[:, :, :, D:D + 1], 1.0)

        # ---- input DMAs: whole chunks alternate between the two HW DGE
        # queues (one shared DGE processor serves all queues; big descriptors
        # on 2 queues empirically hit ~230GB/s, finer splits throttle) ----
        for k in range(8):
            qa = nc.sync if k % 2 == 0 else nc.scalar
            qb = nc.scalar if k % 2 == 0 else nc.sync
            qa.dma_start(xT_sb[k][:], xT[k * 128:(k + 1) * 128, :])
            qb.dma_start(wqkv_sb[k][:], wqkv[k * 128:(k + 1) * 128, :])
        wp_sb = []
        for pr in range(2):
            t = wpool.tile([128, C], mmdt, name=f"wp{pr}", tag=f"wp{pr}")
            nc.gpsimd.dma_start(t[:], wp[pr * 128:(pr + 1) * 128, :])
            wp_sb.append(t)

        # ---- phases A+B fused: 4 column passes streaming the x chunks ----
        for t4 in range(4):
            pa = [ps2.tile([128, 1024], f32, name="psA", tag="st")
                  for _ in range(2)]
            psA = [pa[m // 2][:, (m % 2) * 512:(m % 2 + 1) * 512]
                   for m in range(4)]
            pb = [ps1.tile([128, 512], f32, name="psB", tag="ps1")
                  for _ in range(2)]
            pbd = [psd.tile([128, 512], f32, name="psBx", tag="psd")
                   for _ in range(2)]
            psB = [t[:, 0:256] for t in pb] + [t[:, 0:256] for t in pbd]
            tbs = list(range(t4 * 4, t4 * 4 + 4))
            for k in range(8):
                for m in range(4):
                    nc.tensor.matmul(
                        psA[m],
                        wqk_sb[k][:, m * 128:(m + 1) * 128],
                        xT_sb[k][:, t4 * 512:(t4 + 1) * 512],
                        start=(k == 0),
                        stop=(k == 7),
                    )
                for i, tb in enumerate(tbs):
                    nc.tensor.matmul(
                        psB[i],
                        xT_sb[k][:, tb * 128:(tb + 1) * 128],
                        wv_sb[k][:],
                        start=(k == 0),
                        stop=(k == 7),
                    )
            sl = slice(t4 * 512, (t4 + 1) * 512)
            # q: split the pair psum into per-head padded lanes
            for pr in range(2):
                for hh in range(2):
                    r0 = hh * 64
                    nc.scalar.copy(
                        qpad_sb[pr][r0:r0 + 64, hh, sl],
                        psA[pr][r0:r0 + 64, :],
                    )
            for pr in range(2):
                nc.scalar.copy(kT_sb[pr][:, sl], psA[2 + pr])
            for i, tb in enumerate(tbs):
                nc.vector.tensor_copy(
                    vp[:, tb, :, 0:D],
                    psB[i].rearrange("p (h d) -> p h d", h=HPG),
                )

        # ---- phases C/D/E fused into one per-sb streaming pipeline ----
        ytn_sb = []
        for pr in range(2):
            t = arena.tile([128, T], mmdt, name=f"ytn{pr}", tag=f"ytn{pr}", bufs=1)
            ytn_sb.append(t)
        # rt: denominator bounce tiles (ping-pong per t4); rows
        # (pr*2+hh)*4 .. +4 hold head (2pr+hh)'s denominators for that t4
        rt_sb = [
            const.tile([16, 128], f32, name=f"rt{i}", tag=f"rt{i}")
            for i in range(2)
        ]
        rtb_sb = [
            const.tile([16, 128], mmdt, name=f"rtb{i}", tag=f"rtb{i}")
            for i in range(2)
        ]
        # per-head unnormalized y.T; row 64 = softmax denominators
        yt_sb = [
            arena.tile([65, T], f32, name=f"yt{h}", tag=f"yt{h}", bufs=1)
            for h in range(HPG)
        ]
        rrp_sb = [
            const.tile([2, T], mmdt, name=f"rrp{p}", tag=f"rrp{p}")
            for p in range(2)
        ]
        # K=2 indicator: row 0 -> out partitions 0:64, row 1 -> 64:128
        ind2 = const.tile([2, 128], mmdt, name="ind2", tag="ind2")
        nc.vector.memset(ind2[:], 1.0)
        nc.gpsimd.affine_select(
            out=ind2[:], in_=ind2[:],
            compare_op=mybir.AluOpType.is_ge, fill=0.0,
            base=0, pattern=[[1, 128]], channel_multiplier=-64,
        )
        nc.gpsimd.affine_select(
            out=ind2[:], in_=ind2[:],
            compare_op=mybir.AluOpType.is_ge, fill=0.0,
            base=63, pattern=[[-1, 128]], channel_multiplier=64,
        )

        def roles_for(sb):
            roles = []
            for dtk in (-2, -1, 0, 1):
                tkb = 2 * sb + dtk
                if 0 <= tkb:
                    roles.append((tkb, "abcd"[dtk + 2]))
            return roles

        # C score layout per head, 768 cols of the expst tile:
        #   a @ [0:128)    key blk 2sb-2, queries 0:128 of sb, lower-tri keep
        #   b @ [128:384)  key blk 2sb-1, all queries; right half lower-tri
        #   c @ [384:640)  key blk 2sb,   all queries; left half upper-tri
        #   d @ [640:768)  key blk 2sb+1, queries 128:256, upper-tri keep
        # st psum packs one role GROUP for BOTH heads per tile (bank0 = h0,
        # bank1 = h1) so each exp is one strided 2-block activation call:
        #   ab tile: h0 a@0,b@128..384 | h1 a@512,b@640..896
        #   cd tile: h0 c@0..256,d@256..384 | h1 @512..896
        def emit_C(pr, sb):
            heads = (2 * pr, 2 * pr + 1)
            roles = roles_for(sb)
            n = len(roles)
            goff = {"a": (0, 128), "b": (128, 384), "c": (0, 256),
                    "d": (256, 384)}
            eoff = {"a": (0, 128), "b": (128, 384), "c": (384, 640),
                    "d": (640, 768)}
            qoff = {"a": (0, 128), "b": (0, 256), "c": (0, 256),
                    "d": (128, 256)}
            st_ab = (ps2.tile([128, 1024], f32, name="stab", tag="st")
                     if n == 4 else None)
            st_cd = ps2.tile([128, 1024], f32, name="stcd", tag="st")
            grp = {"a": st_ab, "b": st_ab, "c": st_cd, "d": st_cd}
            # K=128 zero-padded scores: lhsT = full kT pair block (the
            # other head's k rows hit qpad's zero rows -> exact 0)
            for i, (tkb, role) in enumerate(roles):
                for hh, h in enumerate(heads):
                    c0, c1 = goff[role]
                    q0, q1 = qoff[role]
                    nc.tensor.matmul(
                        grp[role][:, hh * 512 + c0:hh * 512 + c1],
                        kT_sb[pr][:, tkb * 128:(tkb + 1) * 128],
                        qpad_sb[pr][:, hh, sb * 256 + q0:sb * 256 + q1],
                        start=(i % 2 == 0),
                        stop=(i % 2 == 1 or i == n - 1),
                    )
            ep = expst_pool.tile([128, 2, 768], mmdt, name="expst",
                                 tag="expst")
            # one exp per role group covering both heads (strided 2-block)
            if n == 4:
                nc.scalar.activation(
                    ep[:, :, 0:384],
                    st_ab.rearrange("p (h j) -> p h j", h=2, j=512)[:, :, 0:384],
                    mybir.ActivationFunctionType.Exp,
                    scale=0.125,
                )
            nc.scalar.activation(
                ep[:, :, 384:768],
                st_cd.rearrange("p (h j) -> p h j", h=2, j=512)[:, :, 0:384],
                mybir.ActivationFunctionType.Exp,
                scale=0.125,
            )
            # banded mask: multiply const triangle tiles in after exp
            # (scheduler-balanced across DVE/GpSimd)
            # masks are SBUF-only -> GpSimd (the one engine that cannot
            # touch PSUM), freeing Vector/Scalar for the PSUM evacuations
            for hh in range(2):
                if n == 4:
                    # keep p >= j (memory-window edge) on blocks a@0 and
                    # b_right@256
                    dv = ep[:, hh, 0:512].rearrange(
                        "p (x j) -> p x j", x=2, j=256)[:, :, 0:128]
                    nc.gpsimd.tensor_mul(dv, dv, lo2_view)
                # keep j >= p (causal) on blocks c_left@384 and d@640
                uv = ep[:, hh, 256:768].rearrange(
                    "p (x j) -> p x j", x=2, j=256)[:, :, 128:256]
                nc.gpsimd.tensor_mul(uv, uv, up2_view)
            yts_pair = [
                ps1.tile([128, 512], f32, name="yts", tag="ps1")[:, 0:256]
                for _ in heads
            ]
            order = [r for r in roles if r[1] in "bc"] + [
                r for r in roles if r[1] in "ad"
            ]
            # M=128 padded att@v (zero v columns produce zero rows)
            for j, (tkb, role) in enumerate(order):
                for hh, h in enumerate(heads):
                    c0, c1 = eoff[role]
                    q0, q1 = qoff[role]
                    nc.tensor.matmul(
                        yts_pair[hh][:, q0:q1],
                        vp[:, tkb, h, :],
                        ep[:, hh, c0:c1],
                        start=(j == 0),
                        stop=(j == n - 1),
                    )
            for hh, h in enumerate(heads):
                nc.vector.tensor_copy(
                    yt_sb[h][:, sb * 256:(sb + 1) * 256],
                    yts_pair[hh][0:65, :],
                )

        def emit_D_recip(t4):
            # reciprocal on [1, T] is ~us on one DVE lane; bounce the t4's
            # denom rows through a [16, 128] tile with tiny SBUF->SBUF DMAs
            cs = slice(t4 * 512, (t4 + 1) * 512)
            rt = rt_sb[t4 % 2]
            rtb = rtb_sb[t4 % 2]
            for h in range(HPG):
                r0 = h * 4
                nc.gpsimd.dma_start(rt[r0:r0 + 4, :], yt_sb[h][64:65, cs])
            with nc.allow_low_precision(reason="softmax denom reciprocal"):
                nc.vector.reciprocal(rtb[0:16, :], rt[0:16, :])
            # [8,128] rows map linearly onto [2, 512]: one DMA per pair
            # (gpsimd queue: keeps these off the store queues' line)
            for pr in range(2):
                nc.gpsimd.dma_start(
                    rrp_sb[pr][0:2, cs], rtb[pr * 8:(pr + 1) * 8, :]
                )

        def emit_D_norm(pr, t4):
            sl = slice(t4 * 512, (t4 + 1) * 512)
            # pair-stacked broadcast: one K=2 matmul broadcasts both heads'
            # reciprocal rows across partitions, then per-head multiplies
            bc = psd.tile([128, 512], f32, name="bc", tag="psd")
            nc.tensor.matmul(
                bc[:], ind2[:, :], rrp_sb[pr][:, sl],
                start=True, stop=True,
            )
            for hh in range(2):
                r0 = hh * 64
                nc.vector.tensor_mul(
                    ytn_sb[pr][r0:r0 + 64, sl],
                    yt_sb[2 * pr + hh][0:64, sl],
                    bc[r0:r0 + 64, :],
                )

        # ---- phase E: partial projection out = y_g @ W_p[g] (K=128 pairs) --
        store_q = [nc.sync, nc.scalar]

        def emit_E(tbs):
            for tb in tbs:
                ob = outsb_pool.tile([128, 1024], mmdt, name="outsb",
                                     tag="outsb")
                for nh in range(2):
                    ps = psd.tile([128, 512], f32, name="psE", tag="psd")
                    for pr in range(2):
                        nc.tensor.matmul(
                            ps[:],
                            ytn_sb[pr][:, tb * 128:(tb + 1) * 128],
                            wp_sb[pr][:, nh * 512:(nh + 1) * 512],
                            start=(pr == 0),
                            stop=(pr == 1),
                        )
                    if (tb + nh) % 2 == 0:
                        nc.scalar.copy(ob[:, nh * 512:(nh + 1) * 512], ps[:])
                    else:
                        nc.vector.tensor_copy(
                            ob[:, nh * 512:(nh + 1) * 512], ps[:])
                # one [128, 1024] store: 2KB rows, efficient descriptors
                qo = store_q[tb % 2]
                qo.dma_start(out[tb * 128:(tb + 1) * 128, :], ob[:])

        # per quarter-T: attention for both pairs of 2 sbs, then recip,
        # then normalize + project + store those 4 row blocks while the
        # next quarter's attention streams
        for t4 in range(4):
            for sb in (t4 * 2, t4 * 2 + 1):
                emit_C(0, sb)
                emit_C(1, sb)
            emit_D_recip(t4)
            emit_D_norm(0, t4)
            emit_D_norm(1, t4)
            emit_E(range(t4 * 4, t4 * 4 + 4))


def build_program():
    key = "v19"
    if key in _PROGRAM_CACHE:
        return _PROGRAM_CACHE[key]
    import concourse.bacc as bacc
    import concourse.mybir as mybir
    import concourse.tile as tile

    mmdt = mybir.dt.bfloat16
    nc = bacc.Bacc("TRN2", target_bir_lowering=False, debug=False, num_devices=N_CORES)
    xT = nc.dram_tensor("xT", [C, T], mmdt, kind="ExternalInput").ap()
    wqkv = nc.dram_tensor("wqkv", [C, 3 * GC], mmdt, kind="ExternalInput").ap()
    wp = nc.dram_tensor("wp", [GC, C], mmdt, kind="ExternalInput").ap()
    ones_in = nc.dram_tensor("ones_in", [128, 64 + HPG], mmdt,
                             kind="ExternalInput").ap()
    out = nc.dram_tensor("out", [T, C], mmdt, kind="ExternalOutput").ap()
    with tile.TileContext(nc) as tc:
        _emit(tc, nc, xT, wqkv, wp, ones_in, out)
    nc.compile()
    _PROGRAM_CACHE[key] = nc
    return nc


def make_in_maps(x, W_attn, W_proj):
    import ml_dtypes

    x = np.asarray(x, dtype=np.float32)
    W_attn = np.asarray(W_attn, dtype=np.float32)
    W_proj = np.asarray(W_proj, dtype=np.float32)
    cast = lambda a: np.ascontiguousarray(a, dtype=ml_dtypes.bfloat16)
    xTs = [cast(x[b].T) for b in range(B)]
    in_maps = []
    for c in range(N_CORES):
        b, g = divmod(c, G)
        q_cols = W_attn[:, g * GC:(g + 1) * GC]
        k_cols = W_attn[:, C + g * GC:C + (g + 1) * GC]
        v_cols = W_attn[:, 2 * C + g * GC:2 * C + (g + 1) * GC]
        in_maps.append({
            "xT": xTs[b],
            "wqkv": cast(np.concatenate([q_cols, k_cols, v_cols], axis=1)),
            "wp": cast(W_proj[g * GC:(g + 1) * GC, :]),
            "ones_in": cast(np.ones((128, 64 + HPG), dtype=np.float32)),
        })
    return in_maps


def gather(results):
    out = np.zeros((B, T, C), dtype=np.float32)
    for c, res in enumerate(results):
        b = c // G
        out[b] += np.asarray(res["out"], dtype=np.float32)
    return out


def kernel(x, W_attn, W_proj, dtype="bf16", trace=False):
    from concourse import bass_utils

    nc = build_program()
    in_maps = make_in_maps(x, W_attn, W_proj)
    r = bass_utils.run_bass_kernel_spmd(
        nc, in_maps, core_ids=list(range(N_CORES)), trace=trace
    )
    out = gather(r.results)
    if trace:
        kernel.last_results = r
    return out


# revision 41
# speedup vs baseline: 1.1293x; 1.0742x over previous
"""Banded causal self-attention (sparse_attention) for 8 trn2 NeuronCores.

Sharding: tensor-parallel over head groups (4 groups x 4 heads of dim 64)
x data-parallel over batch (2). Core c handles batch c//4, head group c%4.
Each core computes a partial output projection; the host sums the 4 group
partials per batch.

Layout: x is transposed on the host so every matmul on device uses natural
(pre-transposed) operands:
  qkT[512, T]   = W_qk.T @ x.T      (lhsT = W_qk natural, rhs = xT)
  v[T, 256]     = x @ W_v           (lhsT = xT natural,   rhs = W_v)
  scoresT[tk,tq]  computed as lhsT=kT_pair rhs=qpad  (K=128, zero-padded)
  yT+sums       = lhsT=[v|1|0] rhs=exp(scoresT)  (row 64 = softmax denom)
  out[T, C]     = lhsT=yTpair rhs=W_p pair rows (K=128, 2 pairs)
Softmax skips max-subtraction (scores ~ N(0,1) after 1/8 scale; exp is safe
in fp32), so the partition-dim reduction is a fused ones-column in the
att@v matmul.

HAM note: trn2's PE clock gate only un-throttles (1.2 -> 2.4 GHz) when the
MAC-activity over a ~3.4us window is high enough.  K=64 score matmuls and
M=65 att@v matmuls use half the array and left the whole attention phase
cold (measured: 58us at K=4/8).  Fix: q is stored in per-head zero-padded
[128, T] lanes so score matmuls contract K=128 (the other head's k rows
multiply exact zeros), and v is padded to 128 columns so att@v runs M=128.
Same cycle count, full MAC activity, PE stays at 2.4 GHz.

Phases A (qkT) and B (v) are fused into 4 column passes that stream the x
chunks as their DMAs land (x is split across DMA queues by partition
range; weights ride the gpsimd queue chunk-by-chunk), keeping the PE fed
during the load window.
"""

import numpy as np

B, T, C = 2, 2048, 1024
N_HEAD = 16
MEMORY = 256
D = 64           # head dim
G = 4            # head groups (tensor parallel)
HPG = 4          # heads per group
GC = HPG * D     # 256 columns per group
N_CORES = 8
TB = T // 128    # 16 row blocks
SB = T // 256    # 8 query super-blocks
D2 = 128         # padded v columns (64 v + 1 ones + 63 zero)

_PROGRAM_CACHE = {}


def _emit(tc, nc, xT, wqkv, wp, ones_in, out):
    import concourse.mybir as mybir

    f32 = mybir.dt.float32
    mmdt = mybir.dt.bfloat16

    from contextlib import ExitStack

    ctx = ExitStack()
    with ctx:
        const = ctx.enter_context(tc.tile_pool(name="const", bufs=1))
        wpool = ctx.enter_context(tc.tile_pool(name="wpool", bufs=1))
        arena = ctx.enter_context(tc.tile_pool(name="arena", bufs=9))
        qkt_pool = ctx.enter_context(tc.tile_pool(name="qkt", bufs=1))
        vplus_pool = ctx.enter_context(tc.tile_pool(name="vplus", bufs=1))
        expst_pool = ctx.enter_context(tc.tile_pool(name="expst", bufs=4))
        outsb_pool = ctx.enter_context(tc.tile_pool(name="outsb", bufs=4))
        # PSUM (8 banks x 2KB): ps2 = 2x[128,1024] (psA pair / ab+cd score
        # tiles), ps1 = 2x[128,512] (B-chains 0,1 / av yts pair), psd =
        # 2x[128,512] (B-chains 2,3 / bc broadcast + projection psE).
        # yts and bc/psE live in different pools so a reciprocal-latency
        # stall on bc can't head-of-line-block the next quarter's av.
        ps1 = ctx.enter_context(tc.tile_pool(name="ps1", bufs=2, space="PSUM"))
        ps2 = ctx.enter_context(tc.tile_pool(name="ps2", bufs=2, space="PSUM"))
        psd = ctx.enter_context(tc.tile_pool(name="psd", bufs=2, space="PSUM"))

        # ---- constants / masks ----
        # two side-by-side triangular keep masks (multiplied in after exp):
        # lo2: keep p >= j (memory-window edge), up2: keep j >= p (causal)
        lo2 = const.tile([128, 256], mmdt, name="lo2", tag="lo2")
        lo2_view = lo2.rearrange("p (b j) -> p b j", b=2, j=128)
        nc.vector.memset(lo2[:], 1.0)
        nc.gpsimd.affine_select(
            out=lo2_view, in_=lo2_view,
            compare_op=mybir.AluOpType.is_ge, fill=0.0,
            base=0, pattern=[[0, 2], [-1, 128]], channel_multiplier=1,
        )
        up2 = const.tile([128, 256], mmdt, name="up2", tag="up2")
        up2_view = up2.rearrange("p (b j) -> p b j", b=2, j=128)
        nc.vector.memset(up2[:], 1.0)
        nc.gpsimd.affine_select(
            out=up2_view, in_=up2_view,
            compare_op=mybir.AluOpType.is_ge, fill=0.0,
            base=0, pattern=[[0, 2], [1, 128]], channel_multiplier=-1,
        )

        # ---- input tiles ----
        xT_sb, wqkv_sb = [], []
        for k in range(8):
            xT_sb.append(arena.tile([128, T], mmdt, name=f"xT{k}", tag="arena"))
            wqkv_sb.append(wpool.tile([128, 3 * GC], mmdt, name=f"wqkv{k}",
                                      tag=f"wqkv{k}"))
        wqk_sb = [t[:, 0:2 * GC] for t in wqkv_sb]
        wv_sb = [t[:, 2 * GC:3 * GC] for t in wqkv_sb]

        # q in zero-padded per-head lanes: qpad[pr][:, hh, :] holds head
        # (2pr+hh)'s qT rows at partition base hh*64, zeros elsewhere, so
        # score matmuls can contract K=128 (full PE rows -> HAM stays warm)
        qpad_sb = [
            qkt_pool.tile([128, 2, T], mmdt, name=f"qpad{pr}", tag=f"qpad{pr}")
            for pr in range(2)
        ]
        for pr in range(2):
            nc.vector.memset(qpad_sb[pr][0:64, 1, :], 0.0)
            nc.vector.memset(qpad_sb[pr][64:128, 0, :], 0.0)
        # k stays pair-stacked: kT[pr] rows hh*64.. hold head (2pr+hh)'s kT
        kT_sb = [
            qkt_pool.tile([128, T], mmdt, name=f"kT{pr}", tag=f"kT{pr}")
            for pr in range(2)
        ]
        # v for all 16 row blocks; per (tb, h): cols 0:64 v, col 64 ones
        # (softmax denom), cols 65:128 zero (M=128 full-array att@v)
        vp = vplus_pool.tile([128, TB, HPG, D2], mmdt, name="vplus",
                             tag="vplus")
        # whole-tile memset (contiguous, fast), then the ones column
        nc.vector.memset(vp[:], 0.0)
        nc.vector.memset(vp[:, :, :, D:D + 1], 1.0)

        # ---- input DMAs: whole chunks alternate between the two HW DGE
        # queues (one shared DGE processor serves all queues; big descriptors
        # on 2 queues empirically hit ~230GB/s, finer splits throttle) ----
        for k in range(8):
            qa = nc.sync if k % 2 == 0 else nc.scalar
            qb = nc.scalar if k % 2 == 0 else nc.sync
            if k == 0:
                # split the first chunk by columns so the first A matmul
                # (which reads cols 0:512) can start sooner
                qa.dma_start(xT_sb[k][:, 0:512], xT[0:128, 0:512])
                qa.dma_start(xT_sb[k][:, 512:T], xT[0:128, 512:T])
            else:
                qa.dma_start(xT_sb[k][:], xT[k * 128:(k + 1) * 128, :])
            qb.dma_start(wqkv_sb[k][:], wqkv[k * 128:(k + 1) * 128, :])
        wp_sb = []
        for pr in range(2):
            t = wpool.tile([128, C], mmdt, name=f"wp{pr}", tag=f"wp{pr}")
            nc.gpsimd.dma_start(t[:], wp[pr * 128:(pr + 1) * 128, :])
            wp_sb.append(t)

        # ---- phases A+B fused: 4 column passes streaming the x chunks ----
        for t4 in range(4):
            pa = [ps2.tile([128, 1024], f32, name="psA", tag="st")
                  for _ in range(2)]
            psA = [pa[m // 2][:, (m % 2) * 512:(m % 2 + 1) * 512]
                   for m in range(4)]
            pb = [ps1.tile([128, 512], f32, name="psB", tag="ps1")
                  for _ in range(2)]
            pbd = [psd.tile([128, 512], f32, name="psBx", tag="psd")
                   for _ in range(2)]
            psB = [t[:, 0:256] for t in pb] + [t[:, 0:256] for t in pbd]
            tbs = list(range(t4 * 4, t4 * 4 + 4))
            for k in range(8):
                for m in range(4):
                    nc.tensor.matmul(
                        psA[m],
                        wqk_sb[k][:, m * 128:(m + 1) * 128],
                        xT_sb[k][:, t4 * 512:(t4 + 1) * 512],
                        start=(k == 0),
                        stop=(k == 7),
                    )
                for i, tb in enumerate(tbs):
                    nc.tensor.matmul(
                        psB[i],
                        xT_sb[k][:, tb * 128:(tb + 1) * 128],
                        wv_sb[k][:],
                        start=(k == 0),
                        stop=(k == 7),
                    )
            sl = slice(t4 * 512, (t4 + 1) * 512)
            # q: split the pair psum into per-head padded lanes; spread the
            # copies over Scalar+Vector so the psA tiles free up fast
            for pr in range(2):
                for hh in range(2):
                    r0 = hh * 64
                    dst = qpad_sb[pr][r0:r0 + 64, hh, sl]
                    src = psA[pr][r0:r0 + 64, :]
                    if hh == 0:
                        nc.scalar.copy(dst, src)
                    else:
                        nc.vector.tensor_copy(dst, src)
            nc.scalar.copy(kT_sb[0][:, sl], psA[2])
            nc.vector.tensor_copy(kT_sb[1][:, sl], psA[3])
            for i, tb in enumerate(tbs):
                nc.vector.tensor_copy(
                    vp[:, tb, :, 0:D],
                    psB[i].rearrange("p (h d) -> p h d", h=HPG),
                )

        # ---- phases C/D/E fused into one per-sb streaming pipeline ----
        ytn_sb = []
        for pr in range(2):
            t = arena.tile([128, T], mmdt, name=f"ytn{pr}", tag=f"ytn{pr}", bufs=1)
            ytn_sb.append(t)
        # rt: denominator bounce tiles (ping-pong per t4); rows
        # (pr*2+hh)*4 .. +4 hold head (2pr+hh)'s denominators for that t4
        rt_sb = [
            const.tile([16, 128], mmdt, name=f"rt{i}", tag=f"rt{i}")
            for i in range(2)
        ]
        rtb_sb = [
            const.tile([16, 128], mmdt, name=f"rtb{i}", tag=f"rtb{i}")
            for i in range(2)
        ]
        # per-head unnormalized y.T; row 64 = softmax denominators.
        # bf16: halves the PSUM-evacuation cost; ~0.4% rounding is well
        # inside the error budget
        yt_sb = [
            arena.tile([65, T], mmdt, name=f"yt{h}", tag=f"yt{h}", bufs=1)
            for h in range(HPG)
        ]
        rrp_sb = [
            const.tile([2, T], mmdt, name=f"rrp{p}", tag=f"rrp{p}")
            for p in range(2)
        ]
        # K=2 indicator: row 0 -> out partitions 0:64, row 1 -> 64:128
        ind2 = const.tile([2, 128], mmdt, name="ind2", tag="ind2")
        nc.vector.memset(ind2[:], 1.0)
        nc.gpsimd.affine_select(
            out=ind2[:], in_=ind2[:],
            compare_op=mybir.AluOpType.is_ge, fill=0.0,
            base=0, pattern=[[1, 128]], channel_multiplier=-64,
        )
        nc.gpsimd.affine_select(
            out=ind2[:], in_=ind2[:],
            compare_op=mybir.AluOpType.is_ge, fill=0.0,
            base=63, pattern=[[-1, 128]], channel_multiplier=64,
        )

        def roles_for(sb):
            roles = []
            for dtk in (-2, -1, 0, 1):
                tkb = 2 * sb + dtk
                if 0 <= tkb:
                    roles.append((tkb, "abcd"[dtk + 2]))
            return roles

        # C score layout per head, 768 cols of the expst tile:
        #   a @ [0:128)    key blk 2sb-2, queries 0:128 of sb, lower-tri keep
        #   b @ [128:384)  key blk 2sb-1, all queries; right half lower-tri
        #   c @ [384:640)  key blk 2sb,   all queries; left half upper-tri
        #   d @ [640:768)  key blk 2sb+1, queries 128:256, upper-tri keep
        # st psum packs one role GROUP for BOTH heads per tile (bank0 = h0,
        # bank1 = h1) so each exp is one strided 2-block activation call:
        #   ab tile: h0 a@0,b@128..384 | h1 a@512,b@640..896
        #   cd tile: h0 c@0..256,d@256..384 | h1 @512..896
        def emit_C(pr, sb):
            heads = (2 * pr, 2 * pr + 1)
            roles = roles_for(sb)
            n = len(roles)
            goff = {"a": (0, 128), "b": (128, 384), "c": (0, 256),
                    "d": (256, 384)}
            eoff = {"a": (0, 128), "b": (128, 384), "c": (384, 640),
                    "d": (640, 768)}
            qoff = {"a": (0, 128), "b": (0, 256), "c": (0, 256),
                    "d": (128, 256)}
            st_ab = (ps2.tile([128, 1024], f32, name="stab", tag="st")
                     if n == 4 else None)
            st_cd = ps2.tile([128, 1024], f32, name="stcd", tag="st")
            grp = {"a": st_ab, "b": st_ab, "c": st_cd, "d": st_cd}
            # K=128 zero-padded scores: lhsT = full kT pair block (the
            # other head's k rows hit qpad's zero rows -> exact 0)
            for i, (tkb, role) in enumerate(roles):
                for hh, h in enumerate(heads):
                    c0, c1 = goff[role]
                    q0, q1 = qoff[role]
                    nc.tensor.matmul(
                        grp[role][:, hh * 512 + c0:hh * 512 + c1],
                        kT_sb[pr][:, tkb * 128:(tkb + 1) * 128],
                        qpad_sb[pr][:, hh, sb * 256 + q0:sb * 256 + q1],
                        start=(i % 2 == 0),
                        stop=(i % 2 == 1 or i == n - 1),
                    )
            ep = expst_pool.tile([128, 2, 768], mmdt, name="expst",
                                 tag="expst")
            # one exp per role group covering both heads (strided 2-block)
            if n == 4:
                nc.scalar.activation(
                    ep[:, :, 0:384],
                    st_ab.rearrange("p (h j) -> p h j", h=2, j=512)[:, :, 0:384],
                    mybir.ActivationFunctionType.Exp,
                    scale=0.125,
                )
            nc.scalar.activation(
                ep[:, :, 384:768],
                st_cd.rearrange("p (h j) -> p h j", h=2, j=512)[:, :, 0:384],
                mybir.ActivationFunctionType.Exp,
                scale=0.125,
            )
            # banded mask: multiply const triangle tiles in after exp
            # (scheduler-balanced across DVE/GpSimd)
            # masks are SBUF-only: split between GpSimd (otherwise idle,
            # though slower per op) and Vector
            for hh in range(2):
                if n == 4:
                    # keep p >= j (memory-window edge) on blocks a@0 and
                    # b_right@256
                    dv = ep[:, hh, 0:512].rearrange(
                        "p (x j) -> p x j", x=2, j=256)[:, :, 0:128]
                    nc.gpsimd.tensor_mul(dv, dv, lo2_view)
                # keep j >= p (causal) on blocks c_left@384 and d@640
                uv = ep[:, hh, 256:768].rearrange(
                    "p (x j) -> p x j", x=2, j=256)[:, :, 128:256]
                nc.vector.tensor_mul(uv, uv, up2_view)
            yts_pair = [
                ps1.tile([128, 512], f32, name="yts", tag="ps1")[:, 0:256]
                for _ in heads
            ]
            order = [r for r in roles if r[1] in "bc"] + [
                r for r in roles if r[1] in "ad"
            ]
            # M=128 padded att@v (zero v columns produce zero rows)
            for j, (tkb, role) in enumerate(order):
                for hh, h in enumerate(heads):
                    c0, c1 = eoff[role]
                    q0, q1 = qoff[role]
                    nc.tensor.matmul(
                        yts_pair[hh][:, q0:q1],
                        vp[:, tkb, h, :],
                        ep[:, hh, c0:c1],
                        start=(j == 0),
                        stop=(j == n - 1),
                    )
            for hh, h in enumerate(heads):
                nc.vector.tensor_copy(
                    yt_sb[h][:, sb * 256:(sb + 1) * 256],
                    yts_pair[hh][0:65, :],
                )

        def emit_D_recip(t4):
            # reciprocal on [1, T] is ~us on one DVE lane; bounce the t4's
            # denom rows through a [16, 128] tile with tiny SBUF->SBUF DMAs
            cs = slice(t4 * 512, (t4 + 1) * 512)
            rt = rt_sb[t4 % 2]
            rtb = rtb_sb[t4 % 2]
            for h in range(HPG):
                r0 = h * 4
                nc.sync.dma_start(rt[r0:r0 + 4, :], yt_sb[h][64:65, cs])
            with nc.allow_low_precision(reason="softmax denom reciprocal"):
                nc.vector.reciprocal(rtb[0:16, :], rt[0:16, :])
            # [8,128] rows map linearly onto [2, 512]: one DMA per pair
            for pr in range(2):
                nc.sync.dma_start(
                    rrp_sb[pr][0:2, cs], rtb[pr * 8:(pr + 1) * 8, :]
                )

        def emit_D_norm(pr, t4):
            sl = slice(t4 * 512, (t4 + 1) * 512)
            # pair-stacked broadcast: one K=2 matmul broadcasts both heads'
            # reciprocal rows across partitions, then per-head multiplies
            bc = psd.tile([128, 512], f32, name="bc", tag="psd")
            nc.tensor.matmul(
                bc[:], ind2[:, :], rrp_sb[pr][:, sl],
                start=True, stop=True,
            )
            for hh in range(2):
                r0 = hh * 64
                nc.vector.tensor_mul(
                    ytn_sb[pr][r0:r0 + 64, sl],
                    yt_sb[2 * pr + hh][0:64, sl],
                    bc[r0:r0 + 64, :],
                )

        # ---- phase E: partial projection out = y_g @ W_p[g] (K=128 pairs) --
        # stores ride gpsimd/scalar queues; sync stays recip-only in C..E
        # so the tiny reciprocal bounces never queue behind a 256KB store
        store_q = [nc.gpsimd, nc.scalar]

        def emit_E(tbs):
            for tb in tbs:
                ob = outsb_pool.tile([128, 1024], mmdt, name="outsb",
                                     tag="outsb")
                for nh in range(2):
                    ps = psd.tile([128, 512], f32, name="psE", tag="psd")
                    for pr in range(2):
                        nc.tensor.matmul(
                            ps[:],
                            ytn_sb[pr][:, tb * 128:(tb + 1) * 128],
                            wp_sb[pr][:, nh * 512:(nh + 1) * 512],
                            start=(pr == 0),
                            stop=(pr == 1),
                        )
                    if (tb + nh) % 2 == 0:
                        nc.scalar.copy(ob[:, nh * 512:(nh + 1) * 512], ps[:])
                    else:
                        nc.vector.tensor_copy(
                            ob[:, nh * 512:(nh + 1) * 512], ps[:])
                # one [128, 1024] store: 2KB rows, efficient descriptors
                qo = store_q[tb % 2]
                qo.dma_start(out[tb * 128:(tb + 1) * 128, :], ob[:])

        # per quarter-T: attention for both pairs of 2 sbs, then recip,
        # then normalize + project + store those 4 row blocks while the
        # next quarter's attention streams
        for t4 in range(4):
            for sb in (t4 * 2, t4 * 2 + 1):
                emit_C(0, sb)
                emit_C(1, sb)
            emit_D_recip(t4)
            emit_D_norm(0, t4)
            emit_D_norm(1, t4)
            emit_E(range(t4 * 4, t4 * 4 + 4))


def build_program():
    key = "v20"
    if key in _PROGRAM_CACHE:
        return _PROGRAM_CACHE[key]
    import concourse.bacc as bacc
    import concourse.mybir as mybir
    import concourse.tile as tile

    mmdt = mybir.dt.bfloat16
    nc = bacc.Bacc("TRN2", target_bir_lowering=False, debug=False, num_devices=N_CORES)
    xT = nc.dram_tensor("xT", [C, T], mmdt, kind="ExternalInput").ap()
    wqkv = nc.dram_tensor("wqkv", [C, 3 * GC], mmdt, kind="ExternalInput").ap()
    wp = nc.dram_tensor("wp", [GC, C], mmdt, kind="ExternalInput").ap()
    ones_in = nc.dram_tensor("ones_in", [128, 64 + HPG], mmdt,
                             kind="ExternalInput").ap()
    out = nc.dram_tensor("out", [T, C], mmdt, kind="ExternalOutput").ap()
    with tile.TileContext(nc) as tc:
        _emit(tc, nc, xT, wqkv, wp, ones_in, out)
    nc.compile()
    _PROGRAM_CACHE[key] = nc
    return nc


def make_in_maps(x, W_attn, W_proj):
    import ml_dtypes

    x = np.asarray(x, dtype=np.float32)
    W_attn = np.asarray(W_attn, dtype=np.float32)
    W_proj = np.asarray(W_proj, dtype=np.float32)
    cast = lambda a: np.ascontiguousarray(a, dtype=ml_dtypes.bfloat16)
    xTs = [cast(x[b].T) for b in range(B)]
    in_maps = []
    for c in range(N_CORES):
        b, g = divmod(c, G)
        q_cols = W_attn[:, g * GC:(g + 1) * GC]
        k_cols = W_attn[:, C + g * GC:C + (g + 1) * GC]
        v_cols = W_attn[:, 2 * C + g * GC:2 * C + (g + 1) * GC]
        in_maps.append({
            "xT": xTs[b],
            "wqkv": cast(np.concatenate([q_cols, k_cols, v_cols], axis=1)),
            "wp": cast(W_proj[g * GC:(g + 1) * GC, :]),
            "ones_in": cast(np.ones((128, 64 + HPG), dtype=np.float32)),
        })
    return in_maps


def gather(results):
    out = np.zeros((B, T, C), dtype=np.float32)
    for c, res in enumerate(results):
        b = c // G
        out[b] += np.asarray(res["out"], dtype=np.float32)
    return out


def kernel(x, W_attn, W_proj, dtype="bf16", trace=False):
    from concourse import bass_utils

    nc = build_program()
    in_maps = make_in_maps(x, W_attn, W_proj)
    r = bass_utils.run_bass_kernel_spmd(
        nc, in_maps, core_ids=list(range(N_CORES)), trace=trace
    )
    out = gather(r.results)
    if trace:
        kernel.last_results = r
    return out


# revision 48
# speedup vs baseline: 1.1869x; 1.0510x over previous
"""Banded causal self-attention (sparse_attention) for 8 trn2 NeuronCores.

Sharding: tensor-parallel over head groups (4 groups x 4 heads of dim 64)
x data-parallel over batch (2). Core c handles batch c//4, head group c%4.
Each core computes a partial output projection; the host sums the 4 group
partials per batch.

Layout: x is transposed on the host so every matmul on device uses natural
(pre-transposed) operands:
  qkT[512, T]   = W_qk.T @ x.T      (lhsT = W_qk natural, rhs = xT)
  v[T, 256]     = x @ W_v           (lhsT = xT natural,   rhs = W_v)
  scoresT[tk,tq]  computed as lhsT=kT_pair rhs=qpad  (K=128, zero-padded)
  yT+sums       = lhsT=[v|1|0] rhs=exp(scoresT)  (row 64 = softmax denom)
  out[T, C]     = lhsT=yTpair rhs=W_p pair rows (K=128, 2 pairs)
Softmax skips max-subtraction (scores ~ N(0,1) after 1/8 scale; exp is safe
in fp32), so the partition-dim reduction is a fused ones-column in the
att@v matmul.

HAM note: trn2's PE clock gate only un-throttles (1.2 -> 2.4 GHz) when the
MAC-activity over a ~3.4us window is high enough.  K=64 score matmuls and
M=65 att@v matmuls use half the array and left the whole attention phase
cold (measured: 58us at K=4/8).  Fix: q is stored in per-head zero-padded
[128, T] lanes so score matmuls contract K=128 (the other head's k rows
multiply exact zeros), and v is padded to 128 columns so att@v runs M=128.
Same cycle count, full MAC activity, PE stays at 2.4 GHz.

Phases A (qkT) and B (v) are fused into 4 column passes that stream the x
chunks as their DMAs land (x is split across DMA queues by partition
range; weights ride the gpsimd queue chunk-by-chunk), keeping the PE fed
during the load window.
"""

import numpy as np

B, T, C = 2, 2048, 1024
N_HEAD = 16
MEMORY = 256
D = 64           # head dim
G = 4            # head groups (tensor parallel)
HPG = 4          # heads per group
GC = HPG * D     # 256 columns per group
N_CORES = 8
TB = T // 128    # 16 row blocks
SB = T // 256    # 8 query super-blocks
D2 = 128         # padded v columns (64 v + 1 ones + 63 zero)

_PROGRAM_CACHE = {}


def _emit(tc, nc, xT, wqkv, wp, ones_in, out):
    import concourse.mybir as mybir

    f32 = mybir.dt.float32
    mmdt = mybir.dt.bfloat16

    from contextlib import ExitStack

    ctx = ExitStack()
    with ctx:
        const = ctx.enter_context(tc.tile_pool(name="const", bufs=1))
        wpool = ctx.enter_context(tc.tile_pool(name="wpool", bufs=1))
        arena = ctx.enter_context(tc.tile_pool(name="arena", bufs=9))
        qkt_pool = ctx.enter_context(tc.tile_pool(name="qkt", bufs=1))
        vplus_pool = ctx.enter_context(tc.tile_pool(name="vplus", bufs=1))
        expst_pool = ctx.enter_context(tc.tile_pool(name="expst", bufs=4))
        outsb_pool = ctx.enter_context(tc.tile_pool(name="outsb", bufs=4))
        # PSUM (8 banks x 2KB): ps2 = 2x[128,1024] (psA pair / ab+cd score
        # tiles), ps1 = 2x[128,512] (B-chains 0,1 / av yts pair), psd =
        # 2x[128,512] (B-chains 2,3 / bc broadcast + projection psE).
        # yts and bc/psE live in different pools so a reciprocal-latency
        # stall on bc can't head-of-line-block the next quarter's av.
        ps1 = ctx.enter_context(tc.tile_pool(name="ps1", bufs=2, space="PSUM"))
        ps2 = ctx.enter_context(tc.tile_pool(name="ps2", bufs=2, space="PSUM"))
        psd = ctx.enter_context(tc.tile_pool(name="psd", bufs=2, space="PSUM"))

        # ---- constants / masks ----
        # two side-by-side triangular keep masks (multiplied in after exp):
        # lo2: keep p >= j (memory-window edge), up2: keep j >= p (causal)
        lo2 = const.tile([128, 256], mmdt, name="lo2", tag="lo2")
        lo2_view = lo2.rearrange("p (b j) -> p b j", b=2, j=128)
        nc.vector.memset(lo2[:], 1.0)
        nc.gpsimd.affine_select(
            out=lo2_view, in_=lo2_view,
            compare_op=mybir.AluOpType.is_ge, fill=0.0,
            base=0, pattern=[[0, 2], [-1, 128]], channel_multiplier=1,
        )
        up2 = const.tile([128, 256], mmdt, name="up2", tag="up2")
        up2_view = up2.rearrange("p (b j) -> p b j", b=2, j=128)
        nc.vector.memset(up2[:], 1.0)
        nc.gpsimd.affine_select(
            out=up2_view, in_=up2_view,
            compare_op=mybir.AluOpType.is_ge, fill=0.0,
            base=0, pattern=[[0, 2], [1, 128]], channel_multiplier=-1,
        )

        # ---- input tiles ----
        xT_sb, wqkv_sb = [], []
        for k in range(8):
            xT_sb.append(arena.tile([128, T], mmdt, name=f"xT{k}", tag="arena"))
            wqkv_sb.append(wpool.tile([128, 3 * GC], mmdt, name=f"wqkv{k}",
                                      tag=f"wqkv{k}"))
        wqk_sb = [t[:, 0:2 * GC] for t in wqkv_sb]
        wv_sb = [t[:, 2 * GC:3 * GC] for t in wqkv_sb]

        # q in zero-padded per-head lanes: qpad[pr][:, hh, :] holds head
        # (2pr+hh)'s qT rows at partition base hh*64, zeros elsewhere, so
        # score matmuls can contract K=128 (full PE rows -> HAM stays warm)
        qpad_sb = [
            qkt_pool.tile([128, 2, T], mmdt, name=f"qpad{pr}", tag=f"qpad{pr}")
            for pr in range(2)
        ]
        for pr in range(2):
            nc.gpsimd.memset(qpad_sb[pr][0:64, 1, :], 0.0)
            nc.gpsimd.memset(qpad_sb[pr][64:128, 0, :], 0.0)
        # k stays pair-stacked: kT[pr] rows hh*64.. hold head (2pr+hh)'s kT
        kT_sb = [
            qkt_pool.tile([128, T], mmdt, name=f"kT{pr}", tag=f"kT{pr}")
            for pr in range(2)
        ]
        # v for all 16 row blocks; per (tb, h): cols 0:64 v, col 64 ones
        # (softmax denom), cols 65:128 zero (M=128 full-array att@v)
        vp = vplus_pool.tile([128, TB, HPG, D2], mmdt, name="vplus",
                             tag="vplus")
        # whole-tile memset (contiguous, fast), then the ones column
        nc.gpsimd.memset(vp[:], 0.0)
        nc.gpsimd.memset(vp[:, :, :, D:D + 1], 1.0)

        # ---- input DMAs: whole chunks alternate between the two HW DGE
        # queues (one shared DGE processor serves all queues; big descriptors
        # on 2 queues empirically hit ~230GB/s, finer splits throttle) ----
        for k in range(8):
            qa = nc.sync if k % 2 == 0 else nc.scalar
            qb = nc.scalar if k % 2 == 0 else nc.sync
            qa.dma_start(xT_sb[k][:], xT[k * 128:(k + 1) * 128, :])
            qb.dma_start(wqkv_sb[k][:], wqkv[k * 128:(k + 1) * 128, :])
        wp_sb = []
        for pr in range(2):
            t = wpool.tile([128, C], mmdt, name=f"wp{pr}", tag=f"wp{pr}")
            nc.gpsimd.dma_start(t[:], wp[pr * 128:(pr + 1) * 128, :])
            wp_sb.append(t)

        # ---- phases A+B fused: 4 column passes streaming the x chunks ----
        for t4 in range(4):
            pa = [ps2.tile([128, 1024], f32, name="psA", tag="st")
                  for _ in range(2)]
            psA = [pa[m // 2][:, (m % 2) * 512:(m % 2 + 1) * 512]
                   for m in range(4)]
            pb = [ps1.tile([128, 512], f32, name="psB", tag="ps1")
                  for _ in range(2)]
            pbd = [psd.tile([128, 512], f32, name="psBx", tag="psd")
                   for _ in range(2)]
            psB = [t[:, 0:256] for t in pb] + [t[:, 0:256] for t in pbd]
            tbs = list(range(t4 * 4, t4 * 4 + 4))
            # B chains first: their PSUM slots free fast (vp copies), so
            # the next pass's B matmuls cover the q/k copy latency on psA
            for k in range(8):
                for i, tb in enumerate(tbs):
                    nc.tensor.matmul(
                        psB[i],
                        xT_sb[k][:, tb * 128:(tb + 1) * 128],
                        wv_sb[k][:],
                        start=(k == 0),
                        stop=(k == 7),
                    )
                for m in range(4):
                    nc.tensor.matmul(
                        psA[m],
                        wqk_sb[k][:, m * 128:(m + 1) * 128],
                        xT_sb[k][:, t4 * 512:(t4 + 1) * 512],
                        start=(k == 0),
                        stop=(k == 7),
                    )
            sl = slice(t4 * 512, (t4 + 1) * 512)
            # q: split the pair psum into per-head padded lanes; spread the
            # copies over Scalar+Vector so the psA tiles free up fast
            for pr in range(2):
                for hh in range(2):
                    r0 = hh * 64
                    dst = qpad_sb[pr][r0:r0 + 64, hh, sl]
                    src = psA[pr][r0:r0 + 64, :]
                    if hh == 0:
                        nc.scalar.copy(dst, src)
                    else:
                        nc.vector.tensor_copy(dst, src)
            nc.scalar.copy(kT_sb[0][:, sl], psA[2])
            nc.vector.tensor_copy(kT_sb[1][:, sl], psA[3])
            for i, tb in enumerate(tbs):
                nc.vector.tensor_copy(
                    vp[:, tb, :, 0:D],
                    psB[i].rearrange("p (h d) -> p h d", h=HPG),
                )

        # ---- phases C/D/E fused into one per-sb streaming pipeline ----
        ytn_sb = []
        for pr in range(2):
            t = arena.tile([128, T], mmdt, name=f"ytn{pr}", tag=f"ytn{pr}", bufs=1)
            ytn_sb.append(t)
        # rt: denominator bounce tiles (ping-pong per t4); rows
        # (pr*2+hh)*4 .. +4 hold head (2pr+hh)'s denominators for that t4
        rt_sb = [
            const.tile([16, 128], f32, name=f"rt{i}", tag=f"rt{i}")
            for i in range(2)
        ]
        rtb_sb = [
            const.tile([16, 128], mmdt, name=f"rtb{i}", tag=f"rtb{i}")
            for i in range(2)
        ]
        # per-head unnormalized y.T; row 64 = softmax denominators
        yt_sb = [
            arena.tile([65, T], f32, name=f"yt{h}", tag=f"yt{h}", bufs=1)
            for h in range(HPG)
        ]
        rrp_sb = [
            const.tile([2, T], mmdt, name=f"rrp{p}", tag=f"rrp{p}")
            for p in range(2)
        ]
        # K=2 indicator: row 0 -> out partitions 0:64, row 1 -> 64:128
        ind2 = const.tile([2, 128], mmdt, name="ind2", tag="ind2")
        nc.vector.memset(ind2[:], 1.0)
        nc.gpsimd.affine_select(
            out=ind2[:], in_=ind2[:],
            compare_op=mybir.AluOpType.is_ge, fill=0.0,
            base=0, pattern=[[1, 128]], channel_multiplier=-64,
        )
        nc.gpsimd.affine_select(
            out=ind2[:], in_=ind2[:],
            compare_op=mybir.AluOpType.is_ge, fill=0.0,
            base=63, pattern=[[-1, 128]], channel_multiplier=64,
        )

        def roles_for(sb):
            roles = []
            for dtk in (-2, -1, 0, 1):
                tkb = 2 * sb + dtk
                if 0 <= tkb:
                    roles.append((tkb, "abcd"[dtk + 2]))
            return roles

        # C score layout per head, 768 cols of the expst tile:
        #   a @ [0:128)    key blk 2sb-2, queries 0:128 of sb, lower-tri keep
        #   b @ [128:384)  key blk 2sb-1, all queries; right half lower-tri
        #   c @ [384:640)  key blk 2sb,   all queries; left half upper-tri
        #   d @ [640:768)  key blk 2sb+1, queries 128:256, upper-tri keep
        # st psum packs one role GROUP for BOTH heads per tile (bank0 = h0,
        # bank1 = h1) so each exp is one strided 2-block activation call:
        #   ab tile: h0 a@0,b@128..384 | h1 a@512,b@640..896
        #   cd tile: h0 c@0..256,d@256..384 | h1 @512..896
        def emit_C(pr, sb):
            heads = (2 * pr, 2 * pr + 1)
            roles = roles_for(sb)
            n = len(roles)
            goff = {"a": (0, 128), "b": (128, 384), "c": (0, 256),
                    "d": (256, 384)}
            eoff = {"a": (0, 128), "b": (128, 384), "c": (384, 640),
                    "d": (640, 768)}
            qoff = {"a": (0, 128), "b": (0, 256), "c": (0, 256),
                    "d": (128, 256)}
            st_ab = (ps2.tile([128, 1024], f32, name="stab", tag="st")
                     if n == 4 else None)
            st_cd = ps2.tile([128, 1024], f32, name="stcd", tag="st")
            grp = {"a": st_ab, "b": st_ab, "c": st_cd, "d": st_cd}
            # K=128 zero-padded scores: lhsT = full kT pair block (the
            # other head's k rows hit qpad's zero rows -> exact 0)
            for i, (tkb, role) in enumerate(roles):
                for hh, h in enumerate(heads):
                    c0, c1 = goff[role]
                    q0, q1 = qoff[role]
                    nc.tensor.matmul(
                        grp[role][:, hh * 512 + c0:hh * 512 + c1],
                        kT_sb[pr][:, tkb * 128:(tkb + 1) * 128],
                        qpad_sb[pr][:, hh, sb * 256 + q0:sb * 256 + q1],
                        start=(i % 2 == 0),
                        stop=(i % 2 == 1 or i == n - 1),
                    )
            ep = expst_pool.tile([128, 2, 768], mmdt, name="expst",
                                 tag="expst")
            # one exp per role group covering both heads (strided 2-block)
            if n == 4:
                nc.scalar.activation(
                    ep[:, :, 0:384],
                    st_ab.rearrange("p (h j) -> p h j", h=2, j=512)[:, :, 0:384],
                    mybir.ActivationFunctionType.Exp,
                    scale=0.125,
                )
            nc.scalar.activation(
                ep[:, :, 384:768],
                st_cd.rearrange("p (h j) -> p h j", h=2, j=512)[:, :, 0:384],
                mybir.ActivationFunctionType.Exp,
                scale=0.125,
            )
            # banded mask: multiply const triangle tiles in after exp
            # (scheduler-balanced across DVE/GpSimd)
            # masks are SBUF-only: split between GpSimd (otherwise idle,
            # though slower per op) and Vector
            for hh in range(2):
                if n == 4:
                    # keep p >= j (memory-window edge) on blocks a@0 and
                    # b_right@256
                    dv = ep[:, hh, 0:512].rearrange(
                        "p (x j) -> p x j", x=2, j=256)[:, :, 0:128]
                    nc.gpsimd.tensor_mul(dv, dv, lo2_view)
                # keep j >= p (causal) on blocks c_left@384 and d@640
                uv = ep[:, hh, 256:768].rearrange(
                    "p (x j) -> p x j", x=2, j=256)[:, :, 128:256]
                nc.vector.tensor_mul(uv, uv, up2_view)
            yts_pair = [
                ps1.tile([128, 512], f32, name="yts", tag="ps1")[:, 0:256]
                for _ in heads
            ]
            order = [r for r in roles if r[1] in "bc"] + [
                r for r in roles if r[1] in "ad"
            ]
            # M=128 padded att@v (zero v columns produce zero rows)
            for j, (tkb, role) in enumerate(order):
                for hh, h in enumerate(heads):
                    c0, c1 = eoff[role]
                    q0, q1 = qoff[role]
                    nc.tensor.matmul(
                        yts_pair[hh][:, q0:q1],
                        vp[:, tkb, h, :],
                        ep[:, hh, c0:c1],
                        start=(j == 0),
                        stop=(j == n - 1),
                    )
            for hh, h in enumerate(heads):
                dst = yt_sb[h][:, sb * 256:(sb + 1) * 256]
                if hh == 0:
                    nc.vector.tensor_copy(dst, yts_pair[hh][0:65, :])
                else:
                    nc.scalar.copy(dst, yts_pair[hh][0:65, :])

        def emit_D_recip(t4):
            # reciprocal on [1, T] is ~us on one DVE lane; bounce the t4's
            # denom rows through a [16, 128] tile with tiny SBUF->SBUF DMAs
            cs = slice(t4 * 512, (t4 + 1) * 512)
            rt = rt_sb[t4 % 2]
            rtb = rtb_sb[t4 % 2]
            for h in range(HPG):
                r0 = h * 4
                nc.sync.dma_start(rt[r0:r0 + 4, :], yt_sb[h][64:65, cs])
            with nc.allow_low_precision(reason="softmax denom reciprocal"):
                nc.vector.reciprocal(rtb[0:16, :], rt[0:16, :])
            # [8,128] rows map linearly onto [2, 512]: one DMA per pair
            for pr in range(2):
                nc.sync.dma_start(
                    rrp_sb[pr][0:2, cs], rtb[pr * 8:(pr + 1) * 8, :]
                )

        def emit_D_norm(pr, t4):
            sl = slice(t4 * 512, (t4 + 1) * 512)
            # pair-stacked broadcast: one K=2 matmul broadcasts both heads'
            # reciprocal rows across partitions, then per-head multiplies
            bc = psd.tile([128, 512], f32, name="bc", tag="psd")
            nc.tensor.matmul(
                bc[:], ind2[:, :], rrp_sb[pr][:, sl],
                start=True, stop=True,
            )
            for hh in range(2):
                r0 = hh * 64
                nc.vector.tensor_mul(
                    ytn_sb[pr][r0:r0 + 64, sl],
                    yt_sb[2 * pr + hh][0:64, sl],
                    bc[r0:r0 + 64, :],
                )

        # ---- phase E: partial projection out = y_g @ W_p[g] (K=128 pairs) --
        # stores ride gpsimd/scalar queues; sync stays recip-only in C..E
        # so the tiny reciprocal bounces never queue behind a 256KB store
        store_q = [nc.gpsimd, nc.scalar]

        def emit_E(tbs):
            for tb in tbs:
                ob = outsb_pool.tile([128, 1024], mmdt, name="outsb",
                                     tag="outsb")
                for nh in range(2):
                    ps = psd.tile([128, 512], f32, name="psE", tag="psd")
                    for pr in range(2):
                        nc.tensor.matmul(
                            ps[:],
                            ytn_sb[pr][:, tb * 128:(tb + 1) * 128],
                            wp_sb[pr][:, nh * 512:(nh + 1) * 512],
                            start=(pr == 0),
                            stop=(pr == 1),
                        )
                    if (tb + nh) % 2 == 0:
                        nc.scalar.copy(ob[:, nh * 512:(nh + 1) * 512], ps[:])
                    else:
                        nc.vector.tensor_copy(
                            ob[:, nh * 512:(nh + 1) * 512], ps[:])
                # one [128, 1024] store: 2KB rows, efficient descriptors
                qo = store_q[tb % 2]
                qo.dma_start(out[tb * 128:(tb + 1) * 128, :], ob[:])

        # per quarter-T: attention for both pairs of 2 sbs, then recip,
        # then normalize + project + store those 4 row blocks while the
        # next quarter's attention streams
        for t4 in range(4):
            for sb in (t4 * 2, t4 * 2 + 1):
                emit_C(0, sb)
                emit_C(1, sb)
            emit_D_recip(t4)
            emit_D_norm(0, t4)
            emit_D_norm(1, t4)
            emit_E(range(t4 * 4, t4 * 4 + 4))


def build_program():
    key = "v21"
    if key in _PROGRAM_CACHE:
        return _PROGRAM_CACHE[key]
    import concourse.bacc as bacc
    import concourse.mybir as mybir
    import concourse.tile as tile

    mmdt = mybir.dt.bfloat16
    nc = bacc.Bacc("TRN2", target_bir_lowering=False, debug=False, num_devices=N_CORES)
    xT = nc.dram_tensor("xT", [C, T], mmdt, kind="ExternalInput").ap()
    wqkv = nc.dram_tensor("wqkv", [C, 3 * GC], mmdt, kind="ExternalInput").ap()
    wp = nc.dram_tensor("wp", [GC, C], mmdt, kind="ExternalInput").ap()
    ones_in = nc.dram_tensor("ones_in", [128, 64 + HPG], mmdt,
                             kind="ExternalInput").ap()
    out = nc.dram_tensor("out", [T, C], mmdt, kind="ExternalOutput").ap()
    with tile.TileContext(nc) as tc:
        _emit(tc, nc, xT, wqkv, wp, ones_in, out)
    nc.compile()
    _PROGRAM_CACHE[key] = nc
    return nc


def make_in_maps(x, W_attn, W_proj):
    import ml_dtypes

    x = np.asarray(x, dtype=np.float32)
    W_attn = np.asarray(W_attn, dtype=np.float32)
    W_proj = np.asarray(W_proj, dtype=np.float32)
    cast = lambda a: np.ascontiguousarray(a, dtype=ml_dtypes.bfloat16)
    xTs = [cast(x[b].T) for b in range(B)]
    in_maps = []
    for c in range(N_CORES):
        b, g = divmod(c, G)
        q_cols = W_attn[:, g * GC:(g + 1) * GC]
        k_cols = W_attn[:, C + g * GC:C + (g + 1) * GC]
        v_cols = W_attn[:, 2 * C + g * GC:2 * C + (g + 1) * GC]
        in_maps.append({
            "xT": xTs[b],
            "wqkv": cast(np.concatenate([q_cols, k_cols, v_cols], axis=1)),
            "wp": cast(W_proj[g * GC:(g + 1) * GC, :]),
            "ones_in": cast(np.ones((128, 64 + HPG), dtype=np.float32)),
        })
    return in_maps


def gather(results):
    out = np.zeros((B, T, C), dtype=np.float32)
    for c, res in enumerate(results):
        b = c // G
        out[b] += np.asarray(res["out"], dtype=np.float32)
    return out


def kernel(x, W_attn, W_proj, dtype="bf16", trace=False):
    from concourse import bass_utils

    nc = build_program()
    in_maps = make_in_maps(x, W_attn, W_proj)
    r = bass_utils.run_bass_kernel_spmd(
        nc, in_maps, core_ids=list(range(N_CORES)), trace=trace
    )
    out = gather(r.results)
    if trace:
        kernel.last_results = r
    return out
